# revision 1
# baseline (speedup 1.0000x reference)
"""Trainium2 Bass kernel for 4-layer ChebNet GCN (K=3) on 8 NeuronCores.

Self-contained: host-side edge preprocessing (dst-window bucketing, source
sorting into int16-addressable ranges), Bass/Tile graph construction, and
SPMD execution via run_bass_kernel_spmd. See class Builder for the device
algorithm.
"""
"""ChebNet GCN (K=3, 4 layers) as a distributed Bass kernel on 8 TRN2 cores.

Sharding: destination-node rows split across cores. Edges are bucketed by
dst window (128 rows), padded to a uniform number of 128-edge chunks per
window. Per chunk: indirect-DMA gather of source rows (bf16), DVE builds a
weighted one-hot [128 edges x 128 dst] via iota/is_equal/mult, PE matmul
accumulates into the window's PSUM tile. Chebyshev dense matmuls run
node-major with transpose-DMA'd activations as lhsT and resident W^T as rhs;
bias folded in via a ones-row matmul; ACT does relu + bf16 cast. AllGather
moves x1 and layer outputs between cores.
"""
import sys

sys.path.insert(0, "/opt/trn_rl_repo")

import numpy as np
import ml_dtypes

import concourse.bass as bass
import concourse.bacc as bacc
import concourse.mybir as mybir
import concourse.tile as tile
from concourse.vector_clock import ScopedClock

BF16 = ml_dtypes.bfloat16
P = 128


# ---------------------------------------------------------------- tile fix
def _patched_drain_and_barrier(self, tick_clock, wait_clock):
    # This walrus build rejects >1 sem-wait on one instruction ("Too many
    # sync wait commands"); put each tail-drain wait on its own SP NOP.
    nop_inst = self.nc.sync.nop(nofuse=True, hint="tile_drain_waits")
    wait_clock.add_sem_waits(nop_inst.ins, ScopedClock({None: tick_clock.global_clock}))
    si = nop_inst.ins.sync_info
    waits = list(si.on_wait) if si is not None else []
    if si is not None:
        si.on_wait = waits[:1]
    for i in range(1, len(waits)):
        extra = self.nc.sync.nop(nofuse=True, hint=f"tile_drain_waits_{i}")
        extra.ins.sync_info = mybir.SyncInfo(on_wait=[waits[i]], on_update=[])
    self.nc.sync.drain()
    self.nc.all_engine_barrier()
    assert self.sems is not None
    popped = self.nc._tile_sem_poison_stack.pop()
    assert popped is self._sem_poison
    self.nc.clear_and_free_semaphores(list(self.sems.allocated().values()))
    self.nc.all_engine_barrier()


tile.TileContext._drain_and_barrier = _patched_drain_and_barrier


# ---------------------------------------------------------------- host prep
def pick_range(n_full):
    """Largest divisor of n_full that fits int16 indexing (<= 25600)."""
    if n_full <= 32256:
        return n_full
    r = 25600
    while n_full % r != 0:
        r -= 128
    return r


def prep_edges(rows, cols, w, n_cores, local_real, local_pad, n_windows, n_full):
    """Bucket edges by (dst core, dst window, src range), pad each
    (window,range) group to a uniform chunk count G. Sources within a window
    are sorted so each group's indices are range-local (fit int16).

    Returns per-core dict:
      idx16 [128, n_windows*4*G*8] int16 (idx i of group at [i%16, i//16];
            partitions 16.. are zero)
      wv, dstv [128, n_windows*4*G] float32 (edge i at [i%128, i//128])
    plus G.
    """
    rows = np.asarray(rows)
    cols = np.asarray(cols)
    w = np.asarray(w)
    RANGE = pick_range(n_full)
    n_ranges = n_full // RANGE
    assert n_full % RANGE == 0
    core = rows // local_real
    loc = rows - core * local_real
    win = loc // P
    dst_in_win = (loc % P).astype(np.float32)
    src_pad = ((cols // local_real) * local_pad + (cols % local_real)).astype(np.int64)
    rng = src_pad // RANGE

    counts = np.zeros((n_cores, n_windows, n_ranges), dtype=np.int64)
    np.add.at(counts, (core, win, rng), 1)
    G = int(np.ceil(counts.max() / P))
    GP = G * P
    per_core = []
    for m in range(n_cores):
        sel = core == m
        key = win[sel] * np.int64(n_full * 2) + src_pad[sel]
        order = np.argsort(key, kind="stable")
        mwin = win[sel][order]
        mrng = rng[sel][order]
        midx = (src_pad[sel][order] % RANGE).astype(np.int32)
        mw = w[sel][order].astype(np.float32)
        mdst = dst_in_win[sel][order]
        ngroups = n_windows * n_ranges
        idx_arr = np.full((ngroups, GP), -1, dtype=np.int32)
        w_arr = np.zeros((ngroups, GP), dtype=np.float32)
        dst_arr = np.zeros((ngroups, GP), dtype=np.float32)
        gid = mwin * n_ranges + mrng
        # edges are sorted by (win, src) so gid is non-decreasing
        group_counts = counts[m].reshape(-1)
        starts = np.zeros(ngroups + 1, dtype=np.int64)
        np.cumsum(group_counts, out=starts[1:])
        pos = np.arange(len(midx)) - starts[gid]
        idx_arr[gid, pos] = midx
        w_arr[gid, pos] = mw
        dst_arr[gid, pos] = mdst
        # per-group valid-index counts (gather descriptor trimming); the
        # first WARM windows gather the full padded count so every msg
        # pool slot is initialized before any trimmed gather leaves SBUF
        # tails stale (stale x 0 one-hot must not be NaN x 0).
        WARM = 4
        gcnt = group_counts.copy()
        gcnt[gcnt == 0] = 1
        idx_arr[np.arange(ngroups)[gcnt == 1], 0] = np.maximum(
            idx_arr[np.arange(ngroups)[gcnt == 1], 0], 0
        )
        warm = np.zeros(ngroups, dtype=bool)
        warm[: WARM * n_ranges] = True
        gcnt[warm] = GP
        idx_arr[warm] = np.maximum(idx_arr[warm], 0)
        # idx16: per group [16, G*8] with idx i at [i%16, i//16],
        # replicated across the 8 Q7-core partition groups
        a = idx_arr.reshape(ngroups, G * 8, 16)  # [g, col, partition]
        block = a.transpose(2, 0, 1).reshape(16, ngroups * G * 8).astype(np.int16)
        idx16 = np.tile(block, (8, 1))
        # wv/dstv: [p, g*G + c] = edge c*128+p
        def to_dev(arr):
            a2 = arr.reshape(ngroups, G, P).transpose(2, 0, 1).reshape(P, ngroups * G)
            return np.ascontiguousarray(a2.astype(np.float32))

        per_core.append(
            dict(idx16=np.ascontiguousarray(idx16), wv=to_dev(w_arr),
                 dstv=to_dev(dst_arr),
                 gcnt=np.ascontiguousarray(
                     gcnt.reshape(1, ngroups).astype(np.int32)))
        )
    return per_core, G


def prep_weights(W, b, F, H, K=3):
    """W: [H, F*K] (torch-style interleaved k). Returns wt [K*F, H] bf16,
    with the Chebyshev recurrence folded in so the device can use
    y2 = L@x1 directly instead of x2 = 2*L@x1 - x:
      row block 0: (W0 - W2)^T, block 1: W1^T, block 2: (2*W2)^T.
    Bias [1, H] bf16."""
    Wk = W.reshape(H, F, K).transpose(2, 1, 0)  # [k, f, h]
    wt = np.concatenate([Wk[0] - Wk[2], Wk[1], 2.0 * Wk[2]], axis=0)
    return np.ascontiguousarray(wt.astype(BF16)), np.ascontiguousarray(
        b.reshape(1, H).astype(BF16)
    )


# ---------------------------------------------------------------- device
class Builder:
    def __init__(self, nc, tc, cfg):
        self.nc = nc
        self.tc = tc
        self.cfg = cfg
        c = cfg
        self.n_windows = c["local_pad"] // P
        self.G = c["G"]
        self.RANGE = pick_range(c["n_full"])
        self.n_ranges = c["n_full"] // self.RANGE
        WC = self.n_windows * self.n_ranges * self.G
        self.sb = tc.alloc_tile_pool(name="resident", bufs=1)
        self.dram = tc.alloc_tile_pool(name="dram", bufs=1, space="DRAM")
        # edge data residents (weights + dst-in-window); idx16 streamed from DRAM
        self.w_res = self.sb.tile([P, WC], mybir.dt.float32, name="w_res")
        self.dst_res = self.sb.tile([P, WC], mybir.dt.float32, name="dst_res")
        wv_in = nc.dram_tensor("wv", [P, WC], mybir.dt.float32, kind="ExternalInput")
        dst_in = nc.dram_tensor("dstv", [P, WC], mybir.dt.float32, kind="ExternalInput")
        self.idx16_in = nc.dram_tensor(
            "idx16", [P, WC * 8], mybir.dt.int16, kind="ExternalInput"
        )
        NG = self.n_windows * self.n_ranges
        gcnt_in = nc.dram_tensor("gcnt", [1, NG], mybir.dt.int32,
                                 kind="ExternalInput")
        self.cnt_res = self.sb.tile([1, NG], mybir.dt.int32, name="cnt_res")
        nc.sync.dma_start(out=self.cnt_res[:], in_=gcnt_in[:, :])
        self.nregs = [nc.gpsimd.alloc_register(f"gtrim{i}") for i in range(8)]
        if int(os.environ.get("CHEB_PRIVSRC", "0")):
            # probe: private (non-Shared) gather source, garbage contents
            self.priv = self.dram.tile(
                [c["n_full"], 512], mybir.dt.bfloat16, name="privsrc"
            )
        else:
            self.priv = None
        pm = os.environ.get("CHEB_PREP", "0")
        if pm == "4":
            # one sem per Tile DMASW lane, rotated per-prep in emission
            # order to match tile_sem_assignment's next_sw_dma_idx walk
            self.gsem = [nc.alloc_semaphore(f"gsem{q}") for q in range(8)]
        elif int(pm):
            self.gsem = [nc.alloc_semaphore(f"gsem{q}") for q in range(4)]
        else:
            self.gsem = None
        self._prep_i = 0
        nc.sync.dma_start(out=self.w_res[:], in_=wv_in[:, :])
        nc.sync.dma_start(out=self.dst_res[:], in_=dst_in[:, :])
        # iota row tile [128, 128] bf16: value = column index
        iota_i = self.sb.tile([P, P], mybir.dt.int32, name="iota_i")
        nc.gpsimd.iota(iota_i[:], pattern=[[1, P]], base=0, channel_multiplier=0)
        self.iota_bf = self.sb.tile([P, P], mybir.dt.bfloat16, name="iota_bf")
        nc.vector.tensor_copy(self.iota_bf[:], iota_i[:])
        # ones column for bias matmuls: [1, 128] bf16
        self.ones_row = self.sb.tile([1, P], mybir.dt.bfloat16, name="ones_row")
        nc.gpsimd.memset(self.ones_row[:], 1.0)
        # shared pools (one allocation for the whole net: no per-phase
        # SBUF reuse barriers, so adjacent phases pipeline freely)
        self.msgp = tc.alloc_tile_pool(
            name="msgp", bufs=int(os.environ.get("CHEB_MSGBUFS", "9")))
        self.idxp = tc.alloc_tile_pool(name="idxp", bufs=3)
        self.ohp = tc.alloc_tile_pool(name="ohp", bufs=24)
        self.spsp = tc.alloc_tile_pool(name="spsp", bufs=6, space="PSUM")
        self.epp = tc.alloc_tile_pool(name="epp", bufs=6)
        self.atp = tc.alloc_tile_pool(name="atp", bufs=24)
        self.dpsp = tc.alloc_tile_pool(name="dpsp", bufs=2, space="PSUM")
        self.hp = tc.alloc_tile_pool(name="hp", bufs=4)

    def release_pools(self):
        for p in (self.hp, self.dpsp, self.atp, self.epp, self.spsp,
                  self.ohp, self.idxp, self.msgp):
            p.release()

    WQ = (0, 28, 52, 76, 100)   # window boundaries, 512-row aligned quarters

    def spmm(self, src_full, out_loc, F, name, x2_from=None, out_quarters=None):
        """out_loc[d] = sum_e w_e * src_full[idx_e]  for dst windows.
        If x2_from is given (an act-local dram AP), compute instead
        out_loc = 2 * spmm_result - x2_from (the T2 Chebyshev term)."""
        nc, tc = self.nc, self.tc
        G = self.G
        NR = self.n_ranges
        G8 = G * 8
        if True:
            msgp, idxp, ohp, psp, epp = (
                self.msgp, self.idxp, self.ohp, self.spsp, self.epp)

            def body(v):
                psum = psp.tile([P, 512], mybir.dt.float32,
                                name="sp_ps")[:, :F]
                idx_win = idxp.tile([P, NR * G8], mybir.dt.int16, name="sp_iw")
                nc.sync.dma_start(
                    out=idx_win[:],
                    in_=self.idx16_in[:, bass.ds(v * NR * G8, NR * G8)],
                )
                self._qctr = getattr(self, "_qctr", 0) + 1
                if not hasattr(self, "_pending_triggers"):
                    self._pending_triggers = []
                pending_triggers = self._pending_triggers
                for r in range(NR):
                    msg = msgp.tile([P, G * 512], mybir.dt.bfloat16,
                                    name="sp_m")[:, : G * F]
                    if SPMM_MODE != "compute":
                        # --- probe knobs (timing experiments only) ---
                        fdiv = int(os.environ.get("CHEB_GF", "1"))
                        pair = int(os.environ.get("CHEB_GPAIR", "1"))
                        qmod = int(os.environ.get("CHEB_QMOD", "4"))
                        spkt = bool(int(os.environ.get("CHEB_SP", "0")))
                        Fg = F // fdiv
                        nidx = (G * P // pair) // P * P
                        g_out = nidx // P
                        estep = F * pair
                        if self.priv is not None:
                            in_ap = self.priv[
                                r * self.RANGE : (r + 1) * self.RANGE, :Fg
                            ]
                            estep = 512
                        elif pair == 1:
                            in_ap = src_full[r * self.RANGE : (r + 1) * self.RANGE, :Fg]
                        else:
                            # view source as [rows/pair, pair*F]; idx values
                            # stay < RANGE <= rows/pair of the FULL tensor.
                            in_ap = src_full[:, :].rearrange(
                                "(a b) f -> a (b f)", b=pair
                            )
                        qn = (self._qctr * NR + r) % qmod
                        if pair == 1 and fdiv == 1 and self.priv is None:
                            nreg = self.nregs[(self._qctr % 2) * 4 + r]
                            nc.gpsimd.reg_load(
                                nreg,
                                self.cnt_res[0:1, bass.ds(v * NR + r, 1)],
                            )
                            nidx_reg = nreg
                        else:
                            nidx_reg = nidx
                        gkw = dict(
                            out_ap=msg[:, : g_out * Fg * pair].rearrange(
                                "p (g f) -> p g f", g=g_out
                            ),
                            in_ap=in_ap,
                            idxs_ap=idx_win[
                                :, r * G8 : r * G8 + max(1, G8 // pair)
                            ],
                            num_idxs=nidx,
                            num_idxs_reg=nidx_reg,
                            elem_size=Fg * pair,
                            elem_step=estep,
                            single_packet=spkt,
                            queue_num=qn,
                        )
                        if self.gsem is not None:
                            if os.environ.get("CHEB_PREP") == "4":
                                psem = self.gsem[self._prep_i % 8]
                                self._prep_i += 1
                            else:
                                psem = self.gsem[qn]
                            nc.gpsimd.dma_gather(
                                prepare_only=True, sem=psem, **gkw
                            )
                            pending_triggers.append(qn)
                            if os.environ.get("CHEB_PREP", "0") == "1":
                                nc.gpsimd.trigger_dma(count=None, queue_num=qn)
                                pending_triggers.clear()
                        else:
                            nc.gpsimd.dma_gather(**gkw)
                    if SPMM_MODE == "gathernosink":
                        continue
                    if SPMM_MODE == "gather":
                        sink = ohp.tile([P, P], mybir.dt.bfloat16, name="sp_sk")
                        nc.vector.tensor_copy(sink[:], msg[:, 0:P])
                        continue
                    for c in range(G):
                        col_s = bass.ds(v * NR * G + r * G + c, 1)
                        oh = ohp.tile([P, P], mybir.dt.bfloat16, name="sp_oh")
                        nc.vector.tensor_scalar(
                            out=oh[:],
                            in0=self.iota_bf[:],
                            scalar1=self.dst_res[:, col_s],
                            scalar2=self.w_res[:, col_s],
                            op0=mybir.AluOpType.is_equal,
                            op1=mybir.AluOpType.mult,
                        )
                        nc.tensor.matmul(
                            out=psum[:],
                            lhsT=oh[:],
                            rhs=msg[:, c * F : (c + 1) * F],
                            start=(r == 0 and c == 0),
                            stop=(r == NR - 1 and c == G - 1),
                        )
                if os.environ.get("CHEB_PREP", "0") != "3" or self._qctr % 2 == 0:
                    for q in dict.fromkeys(pending_triggers):
                        nc.gpsimd.trigger_dma(count=None, queue_num=q)
                    pending_triggers.clear()
                ysb = epp.tile([P, 512], mybir.dt.bfloat16,
                               name="sp_y")[:, :F]
                if SPMM_MODE in ("gather", "gathernosink"):
                    nc.gpsimd.memset(ysb[:], 0.0)
                elif x2_from is None:
                    nc.scalar.activation(
                        ysb[:], psum[:], mybir.ActivationFunctionType.Copy
                    )
                else:
                    act_t = epp.tile([P, F], mybir.dt.bfloat16, name=f"{name}_a")
                    nc.sync.dma_start(
                        out=act_t[:], in_=x2_from[bass.ds(v * P, P), :]
                    )
                    s2 = epp.tile([P, F], mybir.dt.bfloat16, name=f"{name}_s2")
                    nc.scalar.activation(
                        s2[:], psum[:], mybir.ActivationFunctionType.Copy, scale=2.0
                    )
                    nc.vector.tensor_tensor(
                        out=ysb[:], in0=s2[:], in1=act_t[:],
                        op=mybir.AluOpType.subtract,
                    )
                if out_quarters is None:
                    nc.sync.dma_start(
                        out=out_loc[bass.ds(v * P, P), :], in_=ysb[:]
                    )
                else:
                    qt, w0 = out_quarters[cur_q[0]]
                    nc.sync.dma_start(
                        out=qt[bass.ds((v - w0) * P, P), :], in_=ysb[:]
                    )

            cur_q = [0]
            if out_quarters is None:
                tc.For_i_unrolled(
                    0, self.n_windows, 1, body,
                    max_unroll=int(os.environ.get("CHEB_UNROLL", "4")),
                )
            else:
                for qi in range(4):
                    cur_q[0] = qi
                    w0 = self.WQ[qi]
                    nw = self.WQ[qi + 1] - w0
                    tc.For_i_unrolled(
                        0, nw, 1, lambda u, _w=w0: body(u + _w), max_unroll=2
                    )

    def allgather(self, loc, full, name):
        nc = self.nc
        nc.gpsimd.collective_compute(
            "AllGather",
            mybir.AluOpType.bypass,
            replica_groups=[list(range(self.cfg["n_cores"]))],
            ins=[loc[:, :]],
            outs=[full[:, :]],
        )

    def dense(self, acts, F, H, wt_res, bias_res, out_loc, name, relu, out_f32=False):
        """out_loc[n, h] = relu(sum_k acts[k][n, :] @ wtk + bias).
        acts: list of 3 local dram APs [local_pad, F] bf16 (T0, T1, T2).
        wt_res: resident sbuf tile [3F_pad?, ...] -> here [3F partitions? no:
        wt layout [K*F, H] in DRAM; resident tiles per (k,fchunk) loaded once."""
        nc, tc = self.nc, self.tc
        c = self.cfg
        KF = F // P * 3  # number of 128-row k-chunks total across the 3 terms
        n_groups = c["local_pad"] // 512
        out_dt = mybir.dt.float32 if out_f32 else mybir.dt.bfloat16
        if True:
            atp, psp, hp = self.atp, self.dpsp, self.hp

            GB = (0, 7, 13, 19, 25)   # dense-group quarter boundaries

            def body(g):
                at_tiles = []
                for s in range(3):
                    for k in range(F // P):
                        at = atp.tile([P, 512], mybir.dt.bfloat16, name="dn_at")
                        if isinstance(acts[s], list):
                            q = next(i for i in range(4) if GB[i + 1] > g)
                            src_ap = acts[s][q][
                                bass.ds((g - GB[q]) * 512, 512),
                                k * P : (k + 1) * P,
                            ]
                            nc.sync.dma_start_transpose(out=at[:], in_=src_ap)
                        else:
                            nc.sync.dma_start_transpose(
                                out=at[:],
                                in_=acts[s][
                                    bass.ds(g * 512, 512), k * P : (k + 1) * P
                                ],
                            )
                        at_tiles.append((s, k, at))
                for n in range(4):
                    psum = psp.tile([P, 512], mybir.dt.float32,
                                    name="dn_ps")[:, :H]
                    first = True
                    for s, k, at in at_tiles:
                        nc.tensor.matmul(
                            out=psum[:],
                            lhsT=at[:, n * P : (n + 1) * P],
                            rhs=wt_res[s * (F // P) + k][:],
                            start=first,
                            stop=False,
                        )
                        first = False
                    nc.tensor.matmul(
                        out=psum[:],
                        lhsT=self.ones_row[:],
                        rhs=bias_res[:],
                        start=False,
                        stop=True,
                    )
                    h = hp.tile([P, 512], out_dt, name="dn_h")[:, :H]
                    nc.scalar.activation(
                        h[:],
                        psum[:],
                        mybir.ActivationFunctionType.Relu
                        if relu
                        else mybir.ActivationFunctionType.Copy,
                    )
                    nc.sync.dma_start(
                        out=out_loc[bass.ds(g * 512 + n * P, P), :], in_=h[:]
                    )

            for g_static in range(n_groups):
                body(g_static)

    def load_weights(self, wt_dram, bias_dram, F, H, name):
        """Load [K*F, H] weight into F//P*3 resident sbuf tiles + bias row."""
        nc = self.nc
        tiles = []
        for i in range(3 * F // P):
            t = self.sb.tile([P, H], mybir.dt.bfloat16, name=f"{name}_w{i}")
            nc.sync.dma_start(out=t[:], in_=wt_dram[i * P : (i + 1) * P, :])
            tiles.append(t)
        b = self.sb.tile([1, H], mybir.dt.bfloat16, name=f"{name}_b")
        nc.sync.dma_start(out=b[:], in_=bias_dram[:, :])
        return tiles, b


import os
SKIP = set(os.environ.get("CHEB_SKIP", "").split(","))
SPMM_MODE = os.environ.get("CHEB_SPMM_MODE", "full")


def build(cfg):
    nc = bacc.Bacc(
        "TRN2",
        target_bir_lowering=False,
        debug=False,
        num_devices=cfg["n_cores"],
        num_swdge_queues=max(4, int(os.environ.get("CHEB_QMOD", "4"))),
    )
    F_IN, H, F_OUT = cfg["F_IN"], cfg["H"], cfg["F_OUT"]
    lp, nf = cfg["local_pad"], cfg["n_full"]

    xfull = nc.dram_tensor("xfull", [nf, F_IN], mybir.dt.bfloat16, kind="ExternalInput")
    xloc = nc.dram_tensor("xloc", [lp, F_IN], mybir.dt.bfloat16, kind="ExternalInput")
    wts = {}
    dims = [(F_IN, H), (H, H), (H, H), (H, F_OUT)]
    for i, (F, Ho) in enumerate(dims):
        wts[i] = (
            nc.dram_tensor(f"wt{i}", [3 * F, Ho], mybir.dt.bfloat16, kind="ExternalInput"),
            nc.dram_tensor(f"bias{i}", [1, Ho], mybir.dt.bfloat16, kind="ExternalInput"),
        )
    out_ext = nc.dram_tensor("out", [lp, F_OUT], mybir.dt.float32, kind="ExternalOutput")

    with tile.TileContext(nc) as tc:
        b = Builder(nc, tc, cfg)
        w_res = {i: b.load_weights(wts[i][0], wts[i][1], F, Ho, f"L{i}")
                 for i, (F, Ho) in enumerate(dims)}
        act_full, act_loc = xfull, xloc
        for i, (F, Ho) in enumerate(dims):
            last = i == len(dims) - 1
            x1_loc = b.dram.tile([lp, F], mybir.dt.bfloat16, name=f"x1l{i}")
            x1_full = b.dram.tile(
                [nf, F], mybir.dt.bfloat16, addr_space="Shared", name=f"x1f{i}"
            )
            x2_q = [
                b.dram.tile([(b.WQ[q + 1] - b.WQ[q]) * P, F],
                            mybir.dt.bfloat16, name=f"x2l{i}q{q}")
                for q in range(4)
            ]
            if "spmm" not in SKIP:
                b.spmm(act_full, x1_loc, F, f"spmm1_{i}")
            if "ag" not in SKIP:
                b.allgather(x1_loc, x1_full, f"ag_x1_{i}")
            if "spmm" not in SKIP:
                b.spmm(x1_full, None, F, f"spmm2_{i}",
                       out_quarters=[(x2_q[q], b.WQ[q]) for q in range(4)])
            if last:
                if "dense" not in SKIP:
                    b.dense([act_loc, x1_loc, x2_q], F, Ho, w_res[i][0], w_res[i][1],
                            out_ext, f"dense{i}", relu=False, out_f32=True)
            else:
                h_loc = b.dram.tile([lp, Ho], mybir.dt.bfloat16, name=f"hl{i}")
                h_full = b.dram.tile(
                    [nf, Ho], mybir.dt.bfloat16, addr_space="Shared", name=f"hf{i}"
                )
                if "dense" not in SKIP:
                    b.dense([act_loc, x1_loc, x2_q], F, Ho, w_res[i][0], w_res[i][1],
                            h_loc, f"dense{i}", relu=True)
                if "ag" not in SKIP:
                    b.allgather(h_loc, h_full, f"ag_h_{i}")
                act_full, act_loc = h_full, h_loc
        b.release_pools()
        b.sb.release()
        b.dram.release()
    return nc


# ---------------------------------------------------------------- top level
def run(x, edge_rows, edge_cols, edge_weight, Ws, bs, n_cores=8, trace=False,
        N=None):
    """Ws/bs: lists of 4 (W, b) numpy arrays. Returns [N, F_OUT] f32 and the
    BassKernelResults."""
    from concourse.bass_utils import run_bass_kernel_spmd

    N = x.shape[0] if N is None else N
    F_IN = x.shape[1]
    H = Ws[1].shape[0]
    F_OUT = Ws[3].shape[0]
    assert N % n_cores == 0
    local_real = N // n_cores
    local_pad = ((local_real + 511) // 512) * 512
    n_windows = local_pad // P
    n_full = local_pad * n_cores

    per_core, G = prep_edges(
        edge_rows, edge_cols, edge_weight, n_cores, local_real, local_pad,
        n_windows, local_pad * n_cores
    )
    # padded full x layout
    xp = np.zeros((n_full, F_IN), dtype=BF16)
    xb = x.astype(BF16)
    for m in range(n_cores):
        xp[m * local_pad : m * local_pad + local_real] = xb[
            m * local_real : (m + 1) * local_real
        ]
    dims = [(F_IN, H), (H, H), (H, H), (H, F_OUT)]
    wt_np = {}
    for i, (F, Ho) in enumerate(dims):
        wt, bias = prep_weights(Ws[i], bs[i], F, Ho)
        wt_np[f"wt{i}"] = wt
        wt_np[f"bias{i}"] = bias

    cfg = dict(
        n_cores=n_cores, F_IN=F_IN, H=H, F_OUT=F_OUT,
        local_real=local_real, local_pad=local_pad, n_full=n_full, G=G,
    )
    nc = build(cfg)
    if not nc.is_finalized():
        nc.finalize()
    cfg["nc"] = nc
    in_maps = []
    for m in range(n_cores):
        im = dict(
            xfull=xp,
            xloc=np.ascontiguousarray(xp[m * local_pad : (m + 1) * local_pad]),
            idx16=per_core[m]["idx16"],
            wv=per_core[m]["wv"],
            dstv=per_core[m]["dstv"],
            gcnt=per_core[m]["gcnt"],
            **wt_np,
        )
        in_maps.append(im)
    if trace == "timed":
        import timed_exec

        results, times = timed_exec.timed_run(nc, in_maps, n_cores)
        out = np.concatenate(
            [results[m]["out"][:local_real] for m in range(n_cores)], axis=0
        )
        return out, times
    res = run_bass_kernel_spmd(
        nc, in_maps, core_ids=list(range(n_cores)), trace=trace
    )
    out = np.concatenate(
        [res.results[m]["out"][:local_real] for m in range(n_cores)], axis=0
    )
    return out, res


# ---------------------------------------------------------------- entry

N_NODES = 100000
N_EDGES = 3200000
F_IN_, H_, F_OUT_ = 256, 512, 256


def kernel(x, edge_rows, edge_cols, edge_weight, W1, b1, W2, b2, W3, b3,
           Wout, bout):
    Ws = [np.asarray(W1), np.asarray(W2), np.asarray(W3), np.asarray(Wout)]
    bs = [np.asarray(b1), np.asarray(b2), np.asarray(b3), np.asarray(bout)]
    out, _ = run(
        np.asarray(x), np.asarray(edge_rows), np.asarray(edge_cols),
        np.asarray(edge_weight), Ws, bs, n_cores=8, trace=False,
    )
    return out.astype(np.float32)



# revision 2
# speedup vs baseline: 1.1932x; 1.1932x over previous
"""Trainium2 Bass kernel for 4-layer ChebNet GCN (K=3) on 8 NeuronCores.

Self-contained: host-side edge preprocessing (dst-window bucketing, source
sorting into int16-addressable ranges), Bass/Tile graph construction, and
SPMD execution via run_bass_kernel_spmd. See class Builder for the device
algorithm.
"""
"""ChebNet GCN (K=3, 4 layers) as a distributed Bass kernel on 8 TRN2 cores.

Sharding: destination-node rows split across cores. Edges are bucketed by
dst window (128 rows), padded to a uniform number of 128-edge chunks per
window. Per chunk: indirect-DMA gather of source rows (bf16), DVE builds a
weighted one-hot [128 edges x 128 dst] via iota/is_equal/mult, PE matmul
accumulates into the window's PSUM tile. Chebyshev dense matmuls run
node-major with transpose-DMA'd activations as lhsT and resident W^T as rhs;
bias folded in via a ones-row matmul; ACT does relu + bf16 cast. AllGather
moves x1 and layer outputs between cores.
"""
import sys

sys.path.insert(0, "/opt/trn_rl_repo")

import numpy as np
import ml_dtypes

import concourse.bass as bass
import concourse.bacc as bacc
import concourse.mybir as mybir
import concourse.tile as tile
from concourse.vector_clock import ScopedClock

BF16 = ml_dtypes.bfloat16
P = 128


# ---------------------------------------------------------------- tile fix
def _patched_drain_and_barrier(self, tick_clock, wait_clock):
    # This walrus build rejects >1 sem-wait on one instruction ("Too many
    # sync wait commands"); put each tail-drain wait on its own SP NOP.
    nop_inst = self.nc.sync.nop(nofuse=True, hint="tile_drain_waits")
    wait_clock.add_sem_waits(nop_inst.ins, ScopedClock({None: tick_clock.global_clock}))
    si = nop_inst.ins.sync_info
    waits = list(si.on_wait) if si is not None else []
    if si is not None:
        si.on_wait = waits[:1]
    for i in range(1, len(waits)):
        extra = self.nc.sync.nop(nofuse=True, hint=f"tile_drain_waits_{i}")
        extra.ins.sync_info = mybir.SyncInfo(on_wait=[waits[i]], on_update=[])
    self.nc.sync.drain()
    self.nc.all_engine_barrier()
    assert self.sems is not None
    popped = self.nc._tile_sem_poison_stack.pop()
    assert popped is self._sem_poison
    self.nc.clear_and_free_semaphores(list(self.sems.allocated().values()))
    self.nc.all_engine_barrier()


tile.TileContext._drain_and_barrier = _patched_drain_and_barrier


# ---------------------------------------------------------------- host prep
def pick_range(n_full):
    """Largest divisor of n_full that fits int16 indexing (<= 25600)."""
    if n_full <= 32256:
        return n_full
    r = 25600
    while n_full % r != 0:
        r -= 128
    return r


def prep_edges(rows, cols, w, n_cores, local_real, local_pad, n_windows, n_full):
    """Bucket edges by (dst core, dst window, src range), pad each
    (window,range) group to a uniform chunk count G. Sources within a window
    are sorted so each group's indices are range-local (fit int16).

    Returns per-core dict:
      idx16 [128, n_windows*4*G*8] int16 (idx i of group at [i%16, i//16];
            partitions 16.. are zero)
      wv, dstv [128, n_windows*4*G] float32 (edge i at [i%128, i//128])
    plus G.
    """
    rows = np.asarray(rows)
    cols = np.asarray(cols)
    w = np.asarray(w)
    RANGE = pick_range(n_full)
    n_ranges = n_full // RANGE
    assert n_full % RANGE == 0
    core = rows // local_real
    loc = rows - core * local_real
    win = loc // P
    dst_in_win = (loc % P).astype(np.float32)
    src_pad = ((cols // local_real) * local_pad + (cols % local_real)).astype(np.int64)
    rng = src_pad // RANGE

    counts = np.zeros((n_cores, n_windows, n_ranges), dtype=np.int64)
    np.add.at(counts, (core, win, rng), 1)
    G = int(np.ceil(counts.max() / P))
    GP = G * P
    per_core = []
    for m in range(n_cores):
        sel = core == m
        key = win[sel] * np.int64(n_full * 2) + src_pad[sel]
        order = np.argsort(key, kind="stable")
        mwin = win[sel][order]
        mrng = rng[sel][order]
        midx = (src_pad[sel][order] % RANGE).astype(np.int32)
        mw = w[sel][order].astype(np.float32)
        mdst = dst_in_win[sel][order]
        ngroups = n_windows * n_ranges
        idx_arr = np.full((ngroups, GP), -1, dtype=np.int32)
        w_arr = np.zeros((ngroups, GP), dtype=np.float32)
        dst_arr = np.zeros((ngroups, GP), dtype=np.float32)
        gid = mwin * n_ranges + mrng
        # edges are sorted by (win, src) so gid is non-decreasing
        group_counts = counts[m].reshape(-1)
        starts = np.zeros(ngroups + 1, dtype=np.int64)
        np.cumsum(group_counts, out=starts[1:])
        pos = np.arange(len(midx)) - starts[gid]
        idx_arr[gid, pos] = midx
        w_arr[gid, pos] = mw
        dst_arr[gid, pos] = mdst
        # per-group valid-index counts (gather descriptor trimming); the
        # first WARM windows gather the full padded count so every msg
        # pool slot is initialized before any trimmed gather leaves SBUF
        # tails stale (stale x 0 one-hot must not be NaN x 0).
        WARM = 4
        gcnt = group_counts.copy()
        gcnt[gcnt == 0] = 1
        idx_arr[np.arange(ngroups)[gcnt == 1], 0] = np.maximum(
            idx_arr[np.arange(ngroups)[gcnt == 1], 0], 0
        )
        warm = np.zeros(ngroups, dtype=bool)
        warm[: WARM * n_ranges] = True
        gcnt[warm] = GP
        idx_arr[warm] = np.maximum(idx_arr[warm], 0)
        # idx16: per group [16, G*8] with idx i at [i%16, i//16],
        # replicated across the 8 Q7-core partition groups
        a = idx_arr.reshape(ngroups, G * 8, 16)  # [g, col, partition]
        block = a.transpose(2, 0, 1).reshape(16, ngroups * G * 8).astype(np.int16)
        idx16 = np.tile(block, (8, 1))
        # wv/dstv: [p, g*G + c] = edge c*128+p
        def to_dev(arr):
            a2 = arr.reshape(ngroups, G, P).transpose(2, 0, 1).reshape(P, ngroups * G)
            return np.ascontiguousarray(a2.astype(np.float32))

        per_core.append(
            dict(idx16=np.ascontiguousarray(idx16), wv=to_dev(w_arr),
                 dstv=to_dev(dst_arr),
                 gcnt=np.ascontiguousarray(
                     gcnt.reshape(1, ngroups).astype(np.int32)))
        )
    return per_core, G


def prep_weights(W, b, F, H, K=3):
    """W: [H, F*K] (torch-style interleaved k). Returns wt [K*F, H] bf16,
    with the Chebyshev recurrence folded in so the device can use
    y2 = L@x1 directly instead of x2 = 2*L@x1 - x:
      row block 0: (W0 - W2)^T, block 1: W1^T, block 2: (2*W2)^T.
    Bias [1, H] bf16."""
    Wk = W.reshape(H, F, K).transpose(2, 1, 0)  # [k, f, h]
    wt = np.concatenate([Wk[0] - Wk[2], Wk[1], 2.0 * Wk[2]], axis=0)
    return np.ascontiguousarray(wt.astype(BF16)), np.ascontiguousarray(
        b.reshape(1, H).astype(BF16)
    )


# ---------------------------------------------------------------- device
class Builder:
    def __init__(self, nc, tc, cfg):
        self.nc = nc
        self.tc = tc
        self.cfg = cfg
        c = cfg
        self.n_windows = c["local_pad"] // P
        self.G = c["G"]
        self.RANGE = pick_range(c["n_full"])
        self.n_ranges = c["n_full"] // self.RANGE
        WC = self.n_windows * self.n_ranges * self.G
        self.sb = tc.alloc_tile_pool(name="resident", bufs=1)
        self.dram = tc.alloc_tile_pool(name="dram", bufs=1, space="DRAM")
        # edge data residents (weights + dst-in-window); idx16 streamed from DRAM
        self.w_res = self.sb.tile([P, WC], mybir.dt.float32, name="w_res")
        self.dst_res = self.sb.tile([P, WC], mybir.dt.float32, name="dst_res")
        wv_in = nc.dram_tensor("wv", [P, WC], mybir.dt.float32, kind="ExternalInput")
        dst_in = nc.dram_tensor("dstv", [P, WC], mybir.dt.float32, kind="ExternalInput")
        self.idx16_in = nc.dram_tensor(
            "idx16", [P, WC * 8], mybir.dt.int16, kind="ExternalInput"
        )
        NG = self.n_windows * self.n_ranges
        gcnt_in = nc.dram_tensor("gcnt", [1, NG], mybir.dt.int32,
                                 kind="ExternalInput")
        self.cnt_res = self.sb.tile([1, NG], mybir.dt.int32, name="cnt_res")
        nc.sync.dma_start(out=self.cnt_res[:], in_=gcnt_in[:, :])
        self.nregs = [nc.gpsimd.alloc_register(f"gtrim{i}") for i in range(8)]
        if int(os.environ.get("CHEB_PRIVSRC", "0")):
            # probe: private (non-Shared) gather source, garbage contents
            self.priv = self.dram.tile(
                [c["n_full"], 512], mybir.dt.bfloat16, name="privsrc"
            )
        else:
            self.priv = None
        pm = os.environ.get("CHEB_PREP", "0")
        if pm == "4":
            # one sem per Tile DMASW lane, rotated per-prep in emission
            # order to match tile_sem_assignment's next_sw_dma_idx walk
            self.gsem = [nc.alloc_semaphore(f"gsem{q}") for q in range(8)]
        elif int(pm):
            self.gsem = [nc.alloc_semaphore(f"gsem{q}") for q in range(4)]
        else:
            self.gsem = None
        self._prep_i = 0
        nc.sync.dma_start(out=self.w_res[:], in_=wv_in[:, :])
        nc.sync.dma_start(out=self.dst_res[:], in_=dst_in[:, :])
        # iota row tile [128, 128] bf16: value = column index
        iota_i = self.sb.tile([P, P], mybir.dt.int32, name="iota_i")
        nc.gpsimd.iota(iota_i[:], pattern=[[1, P]], base=0, channel_multiplier=0)
        self.iota_bf = self.sb.tile([P, P], mybir.dt.bfloat16, name="iota_bf")
        nc.vector.tensor_copy(self.iota_bf[:], iota_i[:])
        # ones column for bias matmuls: [1, 128] bf16
        self.ones_row = self.sb.tile([1, P], mybir.dt.bfloat16, name="ones_row")
        nc.gpsimd.memset(self.ones_row[:], 1.0)
        # shared pools (one allocation for the whole net: no per-phase
        # SBUF reuse barriers, so adjacent phases pipeline freely)
        self.msgp = tc.alloc_tile_pool(
            name="msgp", bufs=int(os.environ.get("CHEB_MSGBUFS", "9")))
        self.idxp = tc.alloc_tile_pool(name="idxp", bufs=3)
        self.ohp = tc.alloc_tile_pool(name="ohp", bufs=24)
        self.spsp = tc.alloc_tile_pool(name="spsp", bufs=6, space="PSUM")
        self.epp = tc.alloc_tile_pool(name="epp", bufs=6)
        self.atp = tc.alloc_tile_pool(name="atp", bufs=24)
        self.dpsp = tc.alloc_tile_pool(name="dpsp", bufs=2, space="PSUM")
        self.hp = tc.alloc_tile_pool(name="hp", bufs=4)

    def release_pools(self):
        for p in (self.hp, self.dpsp, self.atp, self.epp, self.spsp,
                  self.ohp, self.idxp, self.msgp):
            p.release()

    WQ = (0, 28, 52, 76, 100)   # window boundaries, 512-row aligned quarters

    def spmm(self, src_full, out_loc, F, name, x2_from=None, out_quarters=None):
        """out_loc[d] = sum_e w_e * src_full[idx_e]  for dst windows.
        If x2_from is given (an act-local dram AP), compute instead
        out_loc = 2 * spmm_result - x2_from (the T2 Chebyshev term)."""
        nc, tc = self.nc, self.tc
        G = self.G
        NR = self.n_ranges
        G8 = G * 8
        if True:
            msgp, idxp, ohp, psp, epp = (
                self.msgp, self.idxp, self.ohp, self.spsp, self.epp)

            def body(v):
                psum = psp.tile([P, 512], mybir.dt.float32,
                                name="sp_ps")[:, :F]
                idx_win = idxp.tile([P, NR * G8], mybir.dt.int16, name="sp_iw")
                nc.sync.dma_start(
                    out=idx_win[:],
                    in_=self.idx16_in[:, bass.ds(v * NR * G8, NR * G8)],
                )
                self._qctr = getattr(self, "_qctr", 0) + 1
                if not hasattr(self, "_pending_triggers"):
                    self._pending_triggers = []
                pending_triggers = self._pending_triggers
                for r in range(NR):
                    msg = msgp.tile([P, G * 512], mybir.dt.bfloat16,
                                    name="sp_m")[:, : G * F]
                    if SPMM_MODE != "compute":
                        # --- probe knobs (timing experiments only) ---
                        fdiv = int(os.environ.get("CHEB_GF", "1"))
                        pair = int(os.environ.get("CHEB_GPAIR", "1"))
                        qmod = int(os.environ.get("CHEB_QMOD", "4"))
                        spkt = bool(int(os.environ.get("CHEB_SP", "0")))
                        Fg = F // fdiv
                        nidx = (G * P // pair) // P * P
                        g_out = nidx // P
                        estep = F * pair
                        if self.priv is not None:
                            in_ap = self.priv[
                                r * self.RANGE : (r + 1) * self.RANGE, :Fg
                            ]
                            estep = 512
                        elif pair == 1:
                            in_ap = src_full[r * self.RANGE : (r + 1) * self.RANGE, :Fg]
                        else:
                            # view source as [rows/pair, pair*F]; idx values
                            # stay < RANGE <= rows/pair of the FULL tensor.
                            in_ap = src_full[:, :].rearrange(
                                "(a b) f -> a (b f)", b=pair
                            )
                        qn = (self._qctr * NR + r) % qmod
                        if pair == 1 and fdiv == 1 and self.priv is None:
                            nreg = self.nregs[(self._qctr % 2) * 4 + r]
                            nc.gpsimd.reg_load(
                                nreg,
                                self.cnt_res[0:1, bass.ds(v * NR + r, 1)],
                            )
                            nidx_reg = nreg
                        else:
                            nidx_reg = nidx
                        gkw = dict(
                            out_ap=msg[:, : g_out * Fg * pair].rearrange(
                                "p (g f) -> p g f", g=g_out
                            ),
                            in_ap=in_ap,
                            idxs_ap=idx_win[
                                :, r * G8 : r * G8 + max(1, G8 // pair)
                            ],
                            num_idxs=nidx,
                            num_idxs_reg=nidx_reg,
                            elem_size=Fg * pair,
                            elem_step=estep,
                            single_packet=spkt,
                            queue_num=qn,
                        )
                        if self.gsem is not None:
                            if os.environ.get("CHEB_PREP") == "4":
                                psem = self.gsem[self._prep_i % 8]
                                self._prep_i += 1
                            else:
                                psem = self.gsem[qn]
                            nc.gpsimd.dma_gather(
                                prepare_only=True, sem=psem, **gkw
                            )
                            pending_triggers.append(qn)
                            if os.environ.get("CHEB_PREP", "0") == "1":
                                nc.gpsimd.trigger_dma(count=None, queue_num=qn)
                                pending_triggers.clear()
                        else:
                            nc.gpsimd.dma_gather(**gkw)
                    if SPMM_MODE == "gathernosink":
                        continue
                    if SPMM_MODE == "gather":
                        sink = ohp.tile([P, P], mybir.dt.bfloat16, name="sp_sk")
                        nc.vector.tensor_copy(sink[:], msg[:, 0:P])
                        continue
                    for c in range(G):
                        col_s = bass.ds(v * NR * G + r * G + c, 1)
                        oh = ohp.tile([P, P], mybir.dt.bfloat16, name="sp_oh")
                        nc.vector.tensor_scalar(
                            out=oh[:],
                            in0=self.iota_bf[:],
                            scalar1=self.dst_res[:, col_s],
                            scalar2=self.w_res[:, col_s],
                            op0=mybir.AluOpType.is_equal,
                            op1=mybir.AluOpType.mult,
                        )
                        nc.tensor.matmul(
                            out=psum[:],
                            lhsT=oh[:],
                            rhs=msg[:, c * F : (c + 1) * F],
                            start=(r == 0 and c == 0),
                            stop=(r == NR - 1 and c == G - 1),
                        )
                if os.environ.get("CHEB_PREP", "0") != "3" or self._qctr % 2 == 0:
                    for q in dict.fromkeys(pending_triggers):
                        nc.gpsimd.trigger_dma(count=None, queue_num=q)
                    pending_triggers.clear()
                ysb = epp.tile([P, 512], mybir.dt.bfloat16,
                               name="sp_y")[:, :F]
                if SPMM_MODE in ("gather", "gathernosink"):
                    nc.gpsimd.memset(ysb[:], 0.0)
                elif x2_from is None:
                    nc.scalar.activation(
                        ysb[:], psum[:], mybir.ActivationFunctionType.Copy
                    )
                else:
                    act_t = epp.tile([P, F], mybir.dt.bfloat16, name=f"{name}_a")
                    nc.sync.dma_start(
                        out=act_t[:], in_=x2_from[bass.ds(v * P, P), :]
                    )
                    s2 = epp.tile([P, F], mybir.dt.bfloat16, name=f"{name}_s2")
                    nc.scalar.activation(
                        s2[:], psum[:], mybir.ActivationFunctionType.Copy, scale=2.0
                    )
                    nc.vector.tensor_tensor(
                        out=ysb[:], in0=s2[:], in1=act_t[:],
                        op=mybir.AluOpType.subtract,
                    )
                if out_quarters is None:
                    nc.sync.dma_start(
                        out=out_loc[bass.ds(v * P, P), :], in_=ysb[:]
                    )
                else:
                    qt, w0 = out_quarters[cur_q[0]]
                    nc.sync.dma_start(
                        out=qt[bass.ds((v - w0) * P, P), :], in_=ysb[:]
                    )

            cur_q = [0]
            if out_quarters is None:
                tc.For_i_unrolled(
                    0, self.n_windows, 1, body,
                    max_unroll=int(os.environ.get("CHEB_UNROLL", "4")),
                )
            else:
                for qi in range(4):
                    cur_q[0] = qi
                    w0 = self.WQ[qi]
                    nw = self.WQ[qi + 1] - w0
                    tc.For_i_unrolled(
                        0, nw, 1, lambda u, _w=w0: body(u + _w),
                        max_unroll=int(os.environ.get("CHEB_UNROLL_Q", "2")),
                    )

    def allgather(self, loc, full, name):
        nc = self.nc
        nc.gpsimd.collective_compute(
            "AllGather",
            mybir.AluOpType.bypass,
            replica_groups=[list(range(self.cfg["n_cores"]))],
            ins=[loc[:, :]],
            outs=[full[:, :]],
        )

    def dense(self, acts, F, H, wt_res, bias_res, out_loc, name, relu, out_f32=False):
        """out_loc[n, h] = relu(sum_k acts[k][n, :] @ wtk + bias).
        acts: list of 3 local dram APs [local_pad, F] bf16 (T0, T1, T2).
        wt_res: resident sbuf tile [3F_pad?, ...] -> here [3F partitions? no:
        wt layout [K*F, H] in DRAM; resident tiles per (k,fchunk) loaded once."""
        nc, tc = self.nc, self.tc
        c = self.cfg
        KF = F // P * 3  # number of 128-row k-chunks total across the 3 terms
        n_groups = c["local_pad"] // 512
        out_dt = mybir.dt.float32 if out_f32 else mybir.dt.bfloat16
        if True:
            atp, psp, hp = self.atp, self.dpsp, self.hp

            GB = (0, 7, 13, 19, 25)   # dense-group quarter boundaries

            def body(g):
                at_tiles = []
                for s in range(3):
                    for k in range(F // P):
                        at = atp.tile([P, 512], mybir.dt.bfloat16, name="dn_at")
                        if isinstance(acts[s], list):
                            q = next(i for i in range(4) if GB[i + 1] > g)
                            src_ap = acts[s][q][
                                bass.ds((g - GB[q]) * 512, 512),
                                k * P : (k + 1) * P,
                            ]
                            nc.sync.dma_start_transpose(out=at[:], in_=src_ap)
                        else:
                            nc.sync.dma_start_transpose(
                                out=at[:],
                                in_=acts[s][
                                    bass.ds(g * 512, 512), k * P : (k + 1) * P
                                ],
                            )
                        at_tiles.append((s, k, at))
                for n in range(4):
                    psum = psp.tile([P, 512], mybir.dt.float32,
                                    name="dn_ps")[:, :H]
                    first = True
                    for s, k, at in at_tiles:
                        nc.tensor.matmul(
                            out=psum[:],
                            lhsT=at[:, n * P : (n + 1) * P],
                            rhs=wt_res[s * (F // P) + k][:],
                            start=first,
                            stop=False,
                        )
                        first = False
                    nc.tensor.matmul(
                        out=psum[:],
                        lhsT=self.ones_row[:],
                        rhs=bias_res[:],
                        start=False,
                        stop=True,
                    )
                    h = hp.tile([P, 512], out_dt, name="dn_h")[:, :H]
                    nc.scalar.activation(
                        h[:],
                        psum[:],
                        mybir.ActivationFunctionType.Relu
                        if relu
                        else mybir.ActivationFunctionType.Copy,
                    )
                    nc.sync.dma_start(
                        out=out_loc[bass.ds(g * 512 + n * P, P), :], in_=h[:]
                    )

            for g_static in range(n_groups):
                body(g_static)

    def load_weights(self, wt_dram, bias_dram, F, H, name):
        """Load [K*F, H] weight into F//P*3 resident sbuf tiles + bias row."""
        nc = self.nc
        tiles = []
        for i in range(3 * F // P):
            t = self.sb.tile([P, H], mybir.dt.bfloat16, name=f"{name}_w{i}")
            nc.sync.dma_start(out=t[:], in_=wt_dram[i * P : (i + 1) * P, :])
            tiles.append(t)
        b = self.sb.tile([1, H], mybir.dt.bfloat16, name=f"{name}_b")
        nc.sync.dma_start(out=b[:], in_=bias_dram[:, :])
        return tiles, b


import os
SKIP = set(os.environ.get("CHEB_SKIP", "").split(","))
SPMM_MODE = os.environ.get("CHEB_SPMM_MODE", "full")


def build(cfg):
    nc = bacc.Bacc(
        "TRN2",
        target_bir_lowering=False,
        debug=False,
        num_devices=cfg["n_cores"],
        num_swdge_queues=max(4, int(os.environ.get("CHEB_QMOD", "4"))),
    )
    F_IN, H, F_OUT = cfg["F_IN"], cfg["H"], cfg["F_OUT"]
    lp, nf = cfg["local_pad"], cfg["n_full"]

    xfull = nc.dram_tensor("xfull", [nf, F_IN], mybir.dt.bfloat16, kind="ExternalInput")
    xloc = nc.dram_tensor("xloc", [lp, F_IN], mybir.dt.bfloat16, kind="ExternalInput")
    wts = {}
    dims = [(F_IN, H), (H, H), (H, H), (H, F_OUT)]
    for i, (F, Ho) in enumerate(dims):
        wts[i] = (
            nc.dram_tensor(f"wt{i}", [3 * F, Ho], mybir.dt.bfloat16, kind="ExternalInput"),
            nc.dram_tensor(f"bias{i}", [1, Ho], mybir.dt.bfloat16, kind="ExternalInput"),
        )
    out_ext = nc.dram_tensor("out", [lp, F_OUT], mybir.dt.float32, kind="ExternalOutput")

    with tile.TileContext(nc) as tc:
        b = Builder(nc, tc, cfg)
        w_res = {i: b.load_weights(wts[i][0], wts[i][1], F, Ho, f"L{i}")
                 for i, (F, Ho) in enumerate(dims)}
        act_full, act_loc = xfull, xloc
        for i, (F, Ho) in enumerate(dims):
            last = i == len(dims) - 1
            x1_loc = b.dram.tile([lp, F], mybir.dt.bfloat16, name=f"x1l{i}")
            x1_full = b.dram.tile(
                [nf, F], mybir.dt.bfloat16, addr_space="Shared", name=f"x1f{i}"
            )
            x2_q = [
                b.dram.tile([(b.WQ[q + 1] - b.WQ[q]) * P, F],
                            mybir.dt.bfloat16, name=f"x2l{i}q{q}")
                for q in range(4)
            ]
            if "spmm" not in SKIP:
                b.spmm(act_full, x1_loc, F, f"spmm1_{i}")
            if "ag" not in SKIP:
                b.allgather(x1_loc, x1_full, f"ag_x1_{i}")
            if "spmm" not in SKIP:
                b.spmm(x1_full, None, F, f"spmm2_{i}",
                       out_quarters=[(x2_q[q], b.WQ[q]) for q in range(4)])
            if last:
                if "dense" not in SKIP:
                    b.dense([act_loc, x1_loc, x2_q], F, Ho, w_res[i][0], w_res[i][1],
                            out_ext, f"dense{i}", relu=False, out_f32=True)
            else:
                h_loc = b.dram.tile([lp, Ho], mybir.dt.bfloat16, name=f"hl{i}")
                h_full = b.dram.tile(
                    [nf, Ho], mybir.dt.bfloat16, addr_space="Shared", name=f"hf{i}"
                )
                if "dense" not in SKIP:
                    b.dense([act_loc, x1_loc, x2_q], F, Ho, w_res[i][0], w_res[i][1],
                            h_loc, f"dense{i}", relu=True)
                if "ag" not in SKIP:
                    b.allgather(h_loc, h_full, f"ag_h_{i}")
                act_full, act_loc = h_full, h_loc
        b.release_pools()
        b.sb.release()
        b.dram.release()
    return nc


# ---------------------------------------------------------------- top level
def run(x, edge_rows, edge_cols, edge_weight, Ws, bs, n_cores=8, trace=False,
        N=None):
    """Ws/bs: lists of 4 (W, b) numpy arrays. Returns [N, F_OUT] f32 and the
    BassKernelResults."""
    from concourse.bass_utils import run_bass_kernel_spmd

    N = x.shape[0] if N is None else N
    F_IN = x.shape[1]
    H = Ws[1].shape[0]
    F_OUT = Ws[3].shape[0]
    assert N % n_cores == 0
    local_real = N // n_cores
    local_pad = ((local_real + 511) // 512) * 512
    n_windows = local_pad // P
    n_full = local_pad * n_cores

    per_core, G = prep_edges(
        edge_rows, edge_cols, edge_weight, n_cores, local_real, local_pad,
        n_windows, local_pad * n_cores
    )
    # padded full x layout
    xp = np.zeros((n_full, F_IN), dtype=BF16)
    xb = x.astype(BF16)
    for m in range(n_cores):
        xp[m * local_pad : m * local_pad + local_real] = xb[
            m * local_real : (m + 1) * local_real
        ]
    dims = [(F_IN, H), (H, H), (H, H), (H, F_OUT)]
    wt_np = {}
    for i, (F, Ho) in enumerate(dims):
        wt, bias = prep_weights(Ws[i], bs[i], F, Ho)
        wt_np[f"wt{i}"] = wt
        wt_np[f"bias{i}"] = bias

    cfg = dict(
        n_cores=n_cores, F_IN=F_IN, H=H, F_OUT=F_OUT,
        local_real=local_real, local_pad=local_pad, n_full=n_full, G=G,
    )
    nc = build(cfg)
    if not nc.is_finalized():
        nc.finalize()
    cfg["nc"] = nc
    in_maps = []
    for m in range(n_cores):
        im = dict(
            xfull=xp,
            xloc=np.ascontiguousarray(xp[m * local_pad : (m + 1) * local_pad]),
            idx16=per_core[m]["idx16"],
            wv=per_core[m]["wv"],
            dstv=per_core[m]["dstv"],
            gcnt=per_core[m]["gcnt"],
            **wt_np,
        )
        in_maps.append(im)
    if trace == "timed":
        import timed_exec

        results, times = timed_exec.timed_run(nc, in_maps, n_cores)
        out = np.concatenate(
            [results[m]["out"][:local_real] for m in range(n_cores)], axis=0
        )
        return out, times
    res = run_bass_kernel_spmd(
        nc, in_maps, core_ids=list(range(n_cores)), trace=trace
    )
    out = np.concatenate(
        [res.results[m]["out"][:local_real] for m in range(n_cores)], axis=0
    )
    return out, res


# ---------------------------------------------------------------- entry

N_NODES = 100000
N_EDGES = 3200000
F_IN_, H_, F_OUT_ = 256, 512, 256


def kernel(x, edge_rows, edge_cols, edge_weight, W1, b1, W2, b2, W3, b3,
           Wout, bout):
    Ws = [np.asarray(W1), np.asarray(W2), np.asarray(W3), np.asarray(Wout)]
    bs = [np.asarray(b1), np.asarray(b2), np.asarray(b3), np.asarray(bout)]
    out, _ = run(
        np.asarray(x), np.asarray(edge_rows), np.asarray(edge_cols),
        np.asarray(edge_weight), Ws, bs, n_cores=8, trace=False,
    )
    return out.astype(np.float32)



# revision 13
# speedup vs baseline: 1.2102x; 1.0142x over previous
"""Trainium2 Bass kernel for 4-layer ChebNet GCN (K=3) on 8 NeuronCores.

Self-contained: host-side edge preprocessing (dst-window bucketing, quarter-
aligned source ranges), Bass/Tile graph construction, SPMD execution via
run_bass_kernel_spmd.

Sharding: destination rows split across cores; each core's 100 dst windows
(128 rows) are grouped into 4 quarters (28/24/24/24 windows). The padded
"full" row layout is quarter-major: range r holds quarter r of every core,
so a quarter-chunked AllGather fills exactly one gather source range and
overlaps the producing phase. Per (window, range) edges are padded to
G_r*128-edge chunks; per chunk an indirect-DMA gather fetches source rows
(bf16), DVE builds a weighted one-hot via iota/is_equal/mult, PE accumulates
into the window's PSUM tile.

Layers 1-3 use the Chebyshev form with recurrence-folded weights (device
computes y2 = L x1; dense blocks are [(W0-W2)^T | W1^T | (2W2)^T]). Layer 4
is restructured project-first: y = h A + L(h B + L(h C)) with A=(W0-W2)^T,
B=W1^T, C=(2W2)^T, so both layer-4 spmms run at width 256 instead of 512;
h A / h B are fused into the spmm PSUMs via identity matmuls.
"""
import os
import sys

sys.path.insert(0, "/opt/trn_rl_repo")

import numpy as np
import ml_dtypes

import concourse.bass as bass
import concourse.bacc as bacc
import concourse.mybir as mybir
import concourse.tile as tile
from concourse.vector_clock import ScopedClock

BF16 = ml_dtypes.bfloat16
P = 128
WQ = (0, 28, 52, 76, 100)          # window quarter boundaries
GB = (0, 7, 13, 19, 25)            # same quarters in 512-row dense groups
NQ = 4


# ---------------------------------------------------------------- tile fix
def _patched_drain_and_barrier(self, tick_clock, wait_clock):
    # This walrus build rejects >1 sem-wait on one instruction ("Too many
    # sync wait commands"); put each tail-drain wait on its own SP NOP.
    nop_inst = self.nc.sync.nop(nofuse=True, hint="tile_drain_waits")
    wait_clock.add_sem_waits(nop_inst.ins, ScopedClock({None: tick_clock.global_clock}))
    si = nop_inst.ins.sync_info
    waits = list(si.on_wait) if si is not None else []
    if si is not None:
        si.on_wait = waits[:1]
    for i in range(1, len(waits)):
        extra = self.nc.sync.nop(nofuse=True, hint=f"tile_drain_waits_{i}")
        extra.ins.sync_info = mybir.SyncInfo(on_wait=[waits[i]], on_update=[])
    self.nc.sync.drain()
    self.nc.all_engine_barrier()
    assert self.sems is not None
    popped = self.nc._tile_sem_poison_stack.pop()
    assert popped is self._sem_poison
    self.nc.clear_and_free_semaphores(list(self.sems.allocated().values()))
    self.nc.all_engine_barrier()


tile.TileContext._drain_and_barrier = _patched_drain_and_barrier


# ---------------------------------------------------------------- host prep
def prep_edges(rows, cols, w, n_cores, local_real, local_pad, n_windows):
    """Bucket edges by (dst core, dst window, src quarter-range), pad each
    (window, range) group to G_r 128-edge chunks. Sources are mapped into the
    quarter-major full layout: range r = [quarter r of core 0 | ... core 7],
    so in-range offsets fit int16.

    Returns per-core dict(idx16, wv, dstv, gcnt) plus G (list of 4).
    """
    rows = np.asarray(rows)
    cols = np.asarray(cols)
    w = np.asarray(w)
    QROWS = np.array([(WQ[i + 1] - WQ[i]) * P for i in range(NQ)])  # per-core
    core = rows // local_real
    loc = rows - core * local_real
    win = loc // P
    dst_in_win = (loc % P).astype(np.float32)
    # source mapping into quarter-major full layout
    sm = cols // local_real
    sl = cols - sm * local_real
    sw = sl // P
    rng = np.digitize(sw, WQ[1:NQ])              # 0..3
    idx_in_rng = sm * QROWS[rng] + (sl - P * np.array(WQ)[rng])
    assert idx_in_rng.max() < 32768

    counts = np.zeros((n_cores, n_windows, NQ), dtype=np.int64)
    np.add.at(counts, (core, win, rng), 1)
    G = [int(np.ceil(counts[:, :, r].max() / P)) for r in range(NQ)]
    PG = np.concatenate([[0], np.cumsum(G)])     # prefix chunks
    SG = int(PG[-1])
    gp = [g * P for g in G]
    per_core = []
    for m in range(n_cores):
        sel = core == m
        key = win[sel] * np.int64(4 * 32768) + rng[sel] * np.int64(32768) + idx_in_rng[sel]
        order = np.argsort(key, kind="stable")
        mwin = win[sel][order]
        mrng = rng[sel][order]
        midx = idx_in_rng[sel][order].astype(np.int32)
        mw = w[sel][order].astype(np.float32)
        mdst = dst_in_win[sel][order]
        ngroups = n_windows * NQ
        # per-group padded slot counts (ragged by range)
        slot = np.array([gp[r] for r in range(NQ)] * n_windows)  # [ngroups]
        slot_start = np.concatenate([[0], np.cumsum(slot)])
        total = int(slot_start[-1])
        idx_arr = np.full(total, -1, dtype=np.int32)
        w_arr = np.zeros(total, dtype=np.float32)
        dst_arr = np.zeros(total, dtype=np.float32)
        gid = mwin * NQ + mrng
        group_counts = counts[m].reshape(-1)
        starts = np.zeros(ngroups + 1, dtype=np.int64)
        np.cumsum(group_counts, out=starts[1:])
        pos = np.arange(len(midx)) - starts[gid] + slot_start[gid]
        idx_arr[pos] = midx
        w_arr[pos] = mw
        dst_arr[pos] = mdst
        # per-group valid-index counts (gather descriptor trimming); msg
        # pool slots are zero-initialized once on device, so trimmed
        # gathers leave finite stale tails. Empty groups gather one row 0.
        gcnt = group_counts.copy()
        gcnt[gcnt == 0] = 1
        fill = np.repeat(np.arange(ngroups), slot)
        need = np.arange(total) < slot_start[fill] + gcnt[fill]
        idx_arr[need & (idx_arr < 0)] = 0
        # idx16 layout: per group [16, gp_r//16] with idx i at [i%16, i//16],
        # replicated across the 8 Q7-core partition groups; groups laid
        # consecutively -> window block width SG*8 int16 cols.
        blocks = []
        for g in range(ngroups):
            a = idx_arr[slot_start[g]:slot_start[g + 1]].reshape(-1, 16).T  # [16, gp/16]
            blocks.append(a)
        block = np.concatenate(blocks, axis=1).astype(np.int16)  # [16, SG*8*n_windows]
        idx16 = np.tile(block, (8, 1))
        # wv/dstv: [128, SG*n_windows], edge i of group g at [i%128, col g + i//128]
        def to_dev(arr):
            cols_ = []
            for g in range(ngroups):
                a = arr[slot_start[g]:slot_start[g + 1]].reshape(-1, P).T  # [128, gp/128]
                cols_.append(a)
            return np.ascontiguousarray(np.concatenate(cols_, axis=1).astype(np.float32))

        per_core.append(
            dict(idx16=np.ascontiguousarray(idx16), wv=to_dev(w_arr),
                 dstv=to_dev(dst_arr),
                 gcnt=np.ascontiguousarray(
                     gcnt.reshape(1, ngroups).astype(np.int32)))
        )
    return per_core, G


def prep_weights(W, b, F, H, K=3):
    """W: [H, F*K] (torch-style interleaved k). Returns wt [K*F, H] bf16 with
    the Chebyshev recurrence folded in: row block 0: (W0 - W2)^T, block 1:
    W1^T, block 2: (2*W2)^T. Bias [1, H] bf16."""
    Wk = W.reshape(H, F, K).transpose(2, 1, 0)  # [k, f, h]
    wt = np.concatenate([Wk[0] - Wk[2], Wk[1], 2.0 * Wk[2]], axis=0)
    return np.ascontiguousarray(wt.astype(BF16)), np.ascontiguousarray(
        b.reshape(1, H).astype(BF16)
    )


# ---------------------------------------------------------------- device
class Builder:
    def __init__(self, nc, tc, cfg):
        self.nc = nc
        self.tc = tc
        self.cfg = cfg
        c = cfg
        self.n_windows = c["local_pad"] // P
        self.G = c["G"]                      # per-range chunk counts
        self.SG = sum(self.G)
        self.PG = [0]
        for g in self.G:
            self.PG.append(self.PG[-1] + g)
        self.QROWS = [(WQ[i + 1] - WQ[i]) * P for i in range(NQ)]
        WC = self.n_windows * self.SG
        self.sb = tc.alloc_tile_pool(name="resident", bufs=1)
        self.dram = tc.alloc_tile_pool(name="dram", bufs=1, space="DRAM")
        # edge data residents (weights + dst-in-window); idx16 streamed
        self.w_res = self.sb.tile([P, WC], mybir.dt.float32, name="w_res")
        self.dst_res = self.sb.tile([P, WC], mybir.dt.float32, name="dst_res")
        wv_in = nc.dram_tensor("wv", [P, WC], mybir.dt.float32, kind="ExternalInput")
        dst_in = nc.dram_tensor("dstv", [P, WC], mybir.dt.float32, kind="ExternalInput")
        self.idx16_in = nc.dram_tensor(
            "idx16", [P, WC * 8], mybir.dt.int16, kind="ExternalInput"
        )
        NG = self.n_windows * NQ
        gcnt_in = nc.dram_tensor("gcnt", [1, NG], mybir.dt.int32,
                                 kind="ExternalInput")
        self.cnt_res = self.sb.tile([1, NG], mybir.dt.int32, name="cnt_res")
        nc.sync.dma_start(out=self.cnt_res[:], in_=gcnt_in[:, :])
        self.nregs = [nc.gpsimd.alloc_register(f"gtrim{i}") for i in range(8)]
        nc.sync.dma_start(out=self.w_res[:], in_=wv_in[:, :])
        nc.sync.dma_start(out=self.dst_res[:], in_=dst_in[:, :])
        # iota row tile [128, 128] bf16: value = column index
        iota_i = self.sb.tile([P, P], mybir.dt.int32, name="iota_i")
        nc.gpsimd.iota(iota_i[:], pattern=[[1, P]], base=0, channel_multiplier=0)
        self.iota_bf = self.sb.tile([P, P], mybir.dt.bfloat16, name="iota_bf")
        nc.vector.tensor_copy(self.iota_bf[:], iota_i[:])
        # per-partition index column [128, 1] for identity construction
        pid_i = self.sb.tile([P, 1], mybir.dt.int32, name="pid_i")
        nc.gpsimd.iota(pid_i[:], pattern=[[0, 1]], base=0, channel_multiplier=1)
        pid_f = self.sb.tile([P, 1], mybir.dt.float32, name="pid_f")
        nc.vector.tensor_copy(pid_f[:], pid_i[:])
        # identity and -0.5*identity [128, 128] bf16
        self.ident = self.sb.tile([P, P], mybir.dt.bfloat16, name="ident")
        nc.vector.tensor_scalar(
            out=self.ident[:], in0=self.iota_bf[:], scalar1=pid_f[:],
            scalar2=None, op0=mybir.AluOpType.is_equal,
        )
        # ones column for bias matmuls: [1, 128] bf16
        self.ones_row = self.sb.tile([1, P], mybir.dt.bfloat16, name="ones_row")
        nc.gpsimd.memset(self.ones_row[:], 1.0)
        # shared pools (one allocation for the whole net: no per-phase
        # SBUF reuse barriers, so adjacent phases pipeline freely)
        gmax = max(self.G)
        msg_bufs = int(os.environ.get("CHEB_MSGBUFS", "7"))
        self.msgp = tc.alloc_tile_pool(name="msgp", bufs=msg_bufs)
        self.gmax = gmax
        # zero-fill every msg slot once: trimmed gathers only overwrite the
        # leading rows of a slot, and PE multiplies the stale tail by a zero
        # one-hot — stale bits must be finite, never uninitialized SBUF.
        for _ in range(msg_bufs):
            t = self.msgp.tile([P, gmax * 512], mybir.dt.bfloat16,
                               name="sp_m")
            nc.gpsimd.memset(t[:], 0.0)
        self.idxp = tc.alloc_tile_pool(name="idxp", bufs=3)
        self.ohp = tc.alloc_tile_pool(name="ohp", bufs=24)
        self.spsp = tc.alloc_tile_pool(name="spsp", bufs=5, space="PSUM")
        self.epp = tc.alloc_tile_pool(name="epp", bufs=6)
        self.frp = tc.alloc_tile_pool(name="frp", bufs=6)
        self.atp = tc.alloc_tile_pool(name="atp", bufs=24)
        self.dpsp = tc.alloc_tile_pool(name="dpsp", bufs=3, space="PSUM")
        self.hp = tc.alloc_tile_pool(name="hp", bufs=6)
        self._gctr = 0

    def release_pools(self):
        for p in (self.hp, self.dpsp, self.atp, self.frp, self.epp, self.spsp,
                  self.ohp, self.idxp, self.msgp):
            p.release()

    def quarter_tensors(self, name, Fo, dt=mybir.dt.bfloat16, shared=False):
        mk = dict(addr_space="Shared") if shared else {}
        n = self.cfg["n_cores"] if shared else 1
        return [
            self.dram.tile([self.QROWS[q] * n, Fo], dt, name=f"{name}q{q}", **mk)
            for q in range(NQ)
        ]

    def allgather(self, loc_q, full_q):
        self.nc.gpsimd.collective_compute(
            "AllGather",
            mybir.AluOpType.bypass,
            replica_groups=[list(range(self.cfg["n_cores"]))],
            ins=[loc_q[:, :]],
            outs=[full_q[:, :]],
        )

    def spmm(self, srcs, F, name, out_quarters=None, out_single=None,
             fuse=None, bias=None, ag=None, out_scale=None, out_f32=False):
        """Per dst window w: psum = sum_e w_e * srcs[r][idx_e] over the 4
        ranges (+ optional fused matmuls), then write
        out = act_copy(psum [* out_scale]).

        srcs: 4 APs (range r rows x F). fuse: list of (lhsT_tile, rows_q)
        where rows_q is a 4-list of local quarter APs or a single AP —
        psum += lhsT^T @ rows[w]. bias: [1, F] tile. ag: callable(q) emitted
        after each quarter. out_scale: scale on the PSUM->SBUF copy."""
        nc = self.nc
        G, PG, SG = self.G, self.PG, self.SG
        SG8 = SG * 8
        n_mm = SG + (len(fuse) if fuse else 0) + (1 if bias is not None else 0)
        msgp, idxp, ohp, psp, epp = (
            self.msgp, self.idxp, self.ohp, self.spsp, self.epp)

        def window(w, qw):
            wl = w - WQ[qw]
            psum = psp.tile([P, 512], mybir.dt.float32, name="sp_ps")[:, :F]
            idx_win = idxp.tile([P, SG8], mybir.dt.int16, name="sp_iw")
            nc.sync.dma_start(
                out=idx_win[:], in_=self.idx16_in[:, bass.ds(w * SG8, SG8)],
            )
            mm = 0
            for r in range(NQ):
                Gr = G[r]
                msg = msgp.tile([P, self.gmax * 512], mybir.dt.bfloat16,
                                name="sp_m")[:, : Gr * F]
                nreg = self.nregs[self._gctr % 8]
                nc.gpsimd.reg_load(
                    nreg, self.cnt_res[0:1, bass.ds(w * NQ + r, 1)],
                )
                nc.gpsimd.dma_gather(
                    out_ap=msg[:].rearrange("p (g f) -> p g f", g=Gr),
                    in_ap=srcs[r],
                    idxs_ap=idx_win[:, PG[r] * 8: PG[r] * 8 + Gr * 8],
                    num_idxs=Gr * P,
                    num_idxs_reg=nreg,
                    elem_size=F,
                    elem_step=F,
                    single_packet=False,
                    queue_num=self._gctr % 4,
                )
                self._gctr += 1
                for c in range(Gr):
                    col_s = bass.ds(w * SG + PG[r] + c, 1)
                    oh = ohp.tile([P, P], mybir.dt.bfloat16, name="sp_oh")
                    nc.vector.tensor_scalar(
                        out=oh[:],
                        in0=self.iota_bf[:],
                        scalar1=self.dst_res[:, col_s],
                        scalar2=self.w_res[:, col_s],
                        op0=mybir.AluOpType.is_equal,
                        op1=mybir.AluOpType.mult,
                    )
                    nc.tensor.matmul(
                        out=psum[:],
                        lhsT=oh[:],
                        rhs=msg[:, c * F: (c + 1) * F],
                        start=(mm == 0),
                        stop=(mm == n_mm - 1),
                    )
                    mm += 1
            if fuse:
                for lhsT, rows_q in fuse:
                    if isinstance(rows_q, list):
                        src_ap = rows_q[qw][bass.ds(wl * P, P), :F]
                    else:
                        src_ap = rows_q[bass.ds(w * P, P), :F]
                    rt = self.frp.tile([P, 512], mybir.dt.bfloat16,
                                       name="sp_fr")[:, :F]
                    nc.sync.dma_start(out=rt[:], in_=src_ap)
                    nc.tensor.matmul(
                        out=psum[:], lhsT=lhsT[:], rhs=rt[:],
                        start=False, stop=(mm == n_mm - 1),
                    )
                    mm += 1
            if bias is not None:
                nc.tensor.matmul(
                    out=psum[:], lhsT=self.ones_row[:], rhs=bias[:],
                    start=False, stop=True,
                )
                mm += 1
            out_dt = mybir.dt.float32 if out_f32 else mybir.dt.bfloat16
            ysb = epp.tile([P, 512], out_dt, name="sp_y")[:, :F]
            kw = dict(scale=out_scale) if out_scale is not None else {}
            nc.scalar.activation(
                ysb[:], psum[:], mybir.ActivationFunctionType.Copy, **kw
            )
            if out_quarters is not None:
                nc.sync.dma_start(
                    out=out_quarters[qw][bass.ds(wl * P, P), :], in_=ysb[:]
                )
            else:
                nc.sync.dma_start(
                    out=out_single[bass.ds(w * P, P), :], in_=ysb[:]
                )

        for qw in range(NQ):
            for w in range(WQ[qw], WQ[qw + 1]):
                window(w, qw)
            if ag is not None:
                ag(qw)

    def dense(self, acts, act_widths, heads, name):
        """For each 512-row group g: load transposed act tiles, then per
        128-row sub-window n accumulate each head's psum over all (s, k)
        chunks (+ bias), apply relu/copy, store.

        acts: list of S sources (4-list of quarter APs or single AP).
        heads: list of dicts: wt (list of [128, Fo] tiles, one per (s, k)),
        bias (tile or None), relu, out_f32, out (4-list quarter APs or
        single AP), ag (callable(q) or None), Fo."""
        nc = self.nc
        atp, psp, hp = self.atp, self.dpsp, self.hp
        n_groups = self.cfg["local_pad"] // 512

        for g in range(n_groups):
            q = next(i for i in range(NQ) if GB[i + 1] > g)
            gl = g - GB[q]
            at_tiles = []
            for s, src in enumerate(acts):
                Fs = act_widths[s]
                for k in range(Fs // P):
                    at = atp.tile([P, 512], mybir.dt.bfloat16, name="dn_at")
                    if isinstance(src, list):
                        src_ap = src[q][bass.ds(gl * 512, 512), k * P:(k + 1) * P]
                    else:
                        src_ap = src[bass.ds(g * 512, 512), k * P:(k + 1) * P]
                    nc.sync.dma_start_transpose(out=at[:], in_=src_ap)
                    at_tiles.append(at)
            for n in range(4):
                outs = []
                for h in heads:
                    Fo = h["Fo"]
                    psum = psp.tile([P, 512], mybir.dt.float32,
                                    name="dn_ps")[:, :Fo]
                    n_mm = len(at_tiles) + (1 if h["bias"] is not None else 0)
                    for i, at in enumerate(at_tiles):
                        nc.tensor.matmul(
                            out=psum[:],
                            lhsT=at[:, n * P:(n + 1) * P],
                            rhs=h["wt"][i][:],
                            start=(i == 0),
                            stop=(i == n_mm - 1),
                        )
                    if h["bias"] is not None:
                        nc.tensor.matmul(
                            out=psum[:], lhsT=self.ones_row[:],
                            rhs=h["bias"][:], start=False, stop=True,
                        )
                    out_dt = (mybir.dt.float32 if h.get("out_f32")
                              else mybir.dt.bfloat16)
                    ht = hp.tile([P, 512], out_dt, name="dn_h")[:, :Fo]
                    nc.scalar.activation(
                        ht[:], psum[:],
                        mybir.ActivationFunctionType.Relu if h["relu"]
                        else mybir.ActivationFunctionType.Copy,
                    )
                    outs.append((h, ht))
                for h, ht in outs:
                    if isinstance(h["out"], list):
                        nc.sync.dma_start(
                            out=h["out"][q][bass.ds((gl * 4 + n) * P, P), :],
                            in_=ht[:],
                        )
                    else:
                        nc.sync.dma_start(
                            out=h["out"][bass.ds((g * 4 + n) * P, P), :],
                            in_=ht[:],
                        )
            if g + 1 in GB:
                for h in heads:
                    if h.get("ag") is not None:
                        h["ag"](q)

    def load_weights(self, wt_dram, bias_dram, F, H, name):
        """Load [K*F, H] weight into 3*F//P resident sbuf tiles + bias row."""
        nc = self.nc
        tiles = []
        for i in range(3 * F // P):
            t = self.sb.tile([P, H], mybir.dt.bfloat16, name=f"{name}_w{i}")
            nc.sync.dma_start(out=t[:], in_=wt_dram[i * P:(i + 1) * P, :])
            tiles.append(t)
        b = self.sb.tile([1, H], mybir.dt.bfloat16, name=f"{name}_b")
        nc.sync.dma_start(out=b[:], in_=bias_dram[:, :])
        return tiles, b


def build(cfg):
    nc = bacc.Bacc(
        "TRN2",
        target_bir_lowering=False,
        debug=False,
        num_devices=cfg["n_cores"],
        num_swdge_queues=4,
    )
    F_IN, H, F_OUT = cfg["F_IN"], cfg["H"], cfg["F_OUT"]
    lp, nf = cfg["local_pad"], cfg["n_full"]

    xfull = nc.dram_tensor("xfull", [nf, F_IN], mybir.dt.bfloat16, kind="ExternalInput")
    xloc = nc.dram_tensor("xloc", [lp, F_IN], mybir.dt.bfloat16, kind="ExternalInput")
    wts = {}
    dims = [(F_IN, H), (H, H), (H, H), (H, F_OUT)]
    for i, (F, Ho) in enumerate(dims):
        wts[i] = (
            nc.dram_tensor(f"wt{i}", [3 * F, Ho], mybir.dt.bfloat16, kind="ExternalInput"),
            nc.dram_tensor(f"bias{i}", [1, Ho], mybir.dt.bfloat16, kind="ExternalInput"),
        )
    out_ext = nc.dram_tensor("out", [lp, F_OUT], mybir.dt.float32, kind="ExternalOutput")

    with tile.TileContext(nc) as tc:
        b = Builder(nc, tc, cfg)
        w_res = {i: b.load_weights(wts[i][0], wts[i][1], F, Ho, f"L{i}")
                 for i, (F, Ho) in enumerate(dims)}
        RS = [0]
        for q in range(NQ):
            RS.append(RS[-1] + b.QROWS[q] * cfg["n_cores"])
        x_srcs = [xfull[RS[r]:RS[r + 1], :] for r in range(NQ)]

        stage = os.environ.get("CHEB_STAGE", "")
        if stage == "sp1":
            b.spmm(x_srcs, F_IN, "dbg_sp1", out_single=out_ext, out_f32=True)
            b.release_pools()
            b.sb.release()
            b.dram.release()
            return nc
        if stage == "sp2":
            x1q = b.quarter_tensors("dx1", F_IN)
            x1f = b.quarter_tensors("dx1f", F_IN, shared=True)
            b.spmm(x_srcs, F_IN, "dbg_sp1", out_quarters=x1q,
                   ag=lambda q: b.allgather(x1q[q], x1f[q]))
            b.spmm([x1f[r][:, :] for r in range(NQ)], F_IN, "dbg_sp2",
                   out_single=out_ext, out_f32=True)
            b.release_pools()
            b.sb.release()
            b.dram.release()
            return nc

        act_srcs, act_loc, act_w = x_srcs, xloc, F_IN
        for i in range(3):
            F, Ho = dims[i]
            x1q = b.quarter_tensors(f"x1l{i}", F)
            x1f = b.quarter_tensors(f"x1f{i}", F, shared=True)
            x2q = b.quarter_tensors(f"x2l{i}", F)
            hq = b.quarter_tensors(f"hl{i}", Ho)
            hf = b.quarter_tensors(f"hf{i}", Ho, shared=True)
            b.spmm(act_srcs, F, f"sp1_{i}", out_quarters=x1q,
                   ag=lambda q, _l=x1q, _f=x1f: b.allgather(_l[q], _f[q]))
            x1_srcs = [x1f[r][:, :] for r in range(NQ)]
            # folded weights: dense consumes y2 = L x1 directly (block 2 is
            # (2 W2)^T and block 0 absorbs -W2), so no 2*y - x combine here.
            b.spmm(x1_srcs, F, f"sp2_{i}", out_quarters=x2q)
            acts_d = [act_loc if i == 0 else act_qlist, x1q, x2q]
            b.dense(acts_d, [F, F, F],
                    [dict(wt=w_res[i][0], bias=w_res[i][1], relu=True,
                          out=hq, Fo=Ho,
                          ag=lambda q, _l=hq, _f=hf: b.allgather(_l[q], _f[q]))],
                    f"dense{i}")
            act_srcs = [hf[r][:, :] for r in range(NQ)]
            act_qlist = hq
            act_w = Ho

        # ---- layer 4, project-first: y = h A + L(h B + L(h C)) ----
        F, Fo = dims[3]          # 512 -> 256
        wt4 = w_res[3][0]        # 12 tiles: A = 0:4, B = 4:8, C = 8:12
        A_t, B_t, C_t = wt4[0:4], wt4[4:8], wt4[8:12]
        qd = b.quarter_tensors("qd", Fo)
        qf = b.quarter_tensors("qf", Fo, shared=True)
        uq = b.quarter_tensors("ul", Fo)
        uf = b.quarter_tensors("uf", Fo, shared=True)
        hBq = b.quarter_tensors("hB", Fo)
        hAq = b.quarter_tensors("hA", Fo)
        b.dense([act_qlist], [F],
                [dict(wt=C_t, bias=None, relu=False, out=qd, Fo=Fo,
                      ag=lambda q: b.allgather(qd[q], qf[q])),
                 dict(wt=B_t, bias=None, relu=False, out=hBq, Fo=Fo),
                 dict(wt=A_t, bias=None, relu=False, out=hAq, Fo=Fo)],
                "proj4")
        q_srcs = [qf[r][:, :] for r in range(NQ)]
        b.spmm(q_srcs, Fo, "sp1_3", out_quarters=uq,
               fuse=[(b.ident, hBq)],
               ag=lambda q: b.allgather(uq[q], uf[q]))
        u_srcs = [uf[r][:, :] for r in range(NQ)]
        b.spmm(u_srcs, Fo, "sp2_3", out_single=out_ext,
               fuse=[(b.ident, hAq)], bias=w_res[3][1], out_f32=True)

        b.release_pools()
        b.sb.release()
        b.dram.release()
    return nc


# ---------------------------------------------------------------- top level
def run(x, edge_rows, edge_cols, edge_weight, Ws, bs, n_cores=8, trace=False):
    """Ws/bs: lists of 4 (W, b) numpy arrays. Returns [N, F_OUT] f32."""
    from concourse.bass_utils import run_bass_kernel_spmd

    N = x.shape[0]
    F_IN = x.shape[1]
    H = Ws[1].shape[0]
    F_OUT = Ws[3].shape[0]
    assert N % n_cores == 0
    local_real = N // n_cores
    local_pad = ((local_real + 511) // 512) * 512
    n_windows = local_pad // P
    n_full = local_pad * n_cores

    per_core, G = prep_edges(
        edge_rows, edge_cols, edge_weight, n_cores, local_real, local_pad,
        n_windows
    )
    # quarter-major padded full x layout
    QROWS = [(WQ[i + 1] - WQ[i]) * P for i in range(NQ)]
    xb = np.asarray(x).astype(BF16)
    xp = np.zeros((n_full, F_IN), dtype=BF16)
    xl = np.zeros((n_cores, local_pad, F_IN), dtype=BF16)
    rs = 0
    for q in range(NQ):
        for m in range(n_cores):
            lo = WQ[q] * P
            hi = min(WQ[q + 1] * P, local_real)
            if hi > lo:
                seg = xb[m * local_real + lo: m * local_real + hi]
                xp[rs + m * QROWS[q]: rs + m * QROWS[q] + (hi - lo)] = seg
        rs += QROWS[q] * n_cores
    for m in range(n_cores):
        xl[m, :local_real] = xb[m * local_real: (m + 1) * local_real]

    dims = [(F_IN, H), (H, H), (H, H), (H, F_OUT)]
    wt_np = {}
    for i, (F, Ho) in enumerate(dims):
        wt, bias = prep_weights(Ws[i], bs[i], F, Ho)
        wt_np[f"wt{i}"] = wt
        wt_np[f"bias{i}"] = bias

    cfg = dict(
        n_cores=n_cores, F_IN=F_IN, H=H, F_OUT=F_OUT,
        local_real=local_real, local_pad=local_pad, n_full=n_full, G=G,
    )
    nc = build(cfg)
    if not nc.is_finalized():
        nc.finalize()
    in_maps = []
    for m in range(n_cores):
        im = dict(
            xfull=xp,
            xloc=np.ascontiguousarray(xl[m]),
            idx16=per_core[m]["idx16"],
            wv=per_core[m]["wv"],
            dstv=per_core[m]["dstv"],
            gcnt=per_core[m]["gcnt"],
            **wt_np,
        )
        in_maps.append(im)
    if trace == "timed":
        import timed_exec

        results, times = timed_exec.timed_run(nc, in_maps, n_cores)
        out = np.concatenate(
            [results[m]["out"][:local_real] for m in range(n_cores)], axis=0
        )
        return out, times
    res = run_bass_kernel_spmd(
        nc, in_maps, core_ids=list(range(n_cores)), trace=trace
    )
    out = np.concatenate(
        [res.results[m]["out"][:local_real] for m in range(n_cores)], axis=0
    )
    return out, res


# ---------------------------------------------------------------- entry
def kernel(x, edge_rows, edge_cols, edge_weight, W1, b1, W2, b2, W3, b3,
           Wout, bout):
    Ws = [np.asarray(W1), np.asarray(W2), np.asarray(W3), np.asarray(Wout)]
    bs = [np.asarray(b1), np.asarray(b2), np.asarray(b3), np.asarray(bout)]
    out, _ = run(
        np.asarray(x), np.asarray(edge_rows), np.asarray(edge_cols),
        np.asarray(edge_weight), Ws, bs, n_cores=8, trace=False,
    )
    return out.astype(np.float32)


# revision 29
# speedup vs baseline: 1.2701x; 1.0495x over previous
"""Trainium2 Bass kernel for 4-layer ChebNet GCN (K=3) on 8 NeuronCores.

Self-contained: host-side edge preprocessing (dst-window bucketing, quarter-
aligned source ranges), Bass/Tile graph construction, SPMD execution via
run_bass_kernel_spmd.

Sharding: destination rows split across cores; each core's 100 dst windows
(128 rows) are grouped into 4 quarters (28/24/24/24 windows). The padded
"full" row layout is quarter-major: range r holds quarter r of every core,
so a quarter-chunked AllGather fills exactly one gather source range and
overlaps the producing phase. Per (window, range) edges are padded to
G_r*128-edge chunks; per chunk an indirect-DMA gather fetches source rows
(bf16), DVE builds a weighted one-hot via iota/is_equal/mult, PE accumulates
into the window's PSUM tile.

Layers 1-3 use the Chebyshev form with recurrence-folded weights (device
computes y2 = L x1; dense blocks are [(W0-W2)^T | W1^T | (2W2)^T]). Layer 4
is restructured project-first: y = h A + L(h B + L(h C)) with A=(W0-W2)^T,
B=W1^T, C=(2W2)^T, so both layer-4 spmms run at width 256 instead of 512;
h A / h B are fused into the spmm PSUMs via identity matmuls.
"""
import os
import sys

sys.path.insert(0, "/opt/trn_rl_repo")

import numpy as np
import ml_dtypes

import concourse.bass as bass
import concourse.bacc as bacc
import concourse.mybir as mybir
import concourse.tile as tile
from concourse.vector_clock import ScopedClock

NQUEUES = int(os.environ.get("CHEB_QUEUES", "4"))

BF16 = ml_dtypes.bfloat16
P = 128
WQ = (0, 28, 52, 76, 100)          # window quarter boundaries
GB = (0, 7, 13, 19, 25)            # same quarters in 512-row dense groups
NQ = 4


# ---------------------------------------------------------------- tile fix
def _patched_drain_and_barrier(self, tick_clock, wait_clock):
    # This walrus build rejects >1 sem-wait on one instruction ("Too many
    # sync wait commands"); put each tail-drain wait on its own SP NOP.
    nop_inst = self.nc.sync.nop(nofuse=True, hint="tile_drain_waits")
    wait_clock.add_sem_waits(nop_inst.ins, ScopedClock({None: tick_clock.global_clock}))
    si = nop_inst.ins.sync_info
    waits = list(si.on_wait) if si is not None else []
    if si is not None:
        si.on_wait = waits[:1]
    for i in range(1, len(waits)):
        extra = self.nc.sync.nop(nofuse=True, hint=f"tile_drain_waits_{i}")
        extra.ins.sync_info = mybir.SyncInfo(on_wait=[waits[i]], on_update=[])
    self.nc.sync.drain()
    self.nc.all_engine_barrier()
    assert self.sems is not None
    popped = self.nc._tile_sem_poison_stack.pop()
    assert popped is self._sem_poison
    self.nc.clear_and_free_semaphores(list(self.sems.allocated().values()))
    self.nc.all_engine_barrier()


tile.TileContext._drain_and_barrier = _patched_drain_and_barrier


# ---------------------------------------------------------------- host prep
def prep_edges(rows, cols, w, n_cores, local_real, local_pad, n_windows):
    """Bucket edges by (dst core, dst window, src quarter-range), pad each
    (window, range) group to G_r 128-edge chunks. Sources are mapped into the
    quarter-major full layout: range r = [quarter r of core 0 | ... core 7],
    so in-range offsets fit int16.

    Returns per-core dict(idx16, wv, dstv, gcnt) plus G (list of 4).
    """
    rows = np.asarray(rows)
    cols = np.asarray(cols)
    w = np.asarray(w)
    QROWS = np.array([(WQ[i + 1] - WQ[i]) * P for i in range(NQ)])  # per-core
    core = rows // local_real
    loc = rows - core * local_real
    win = loc // P
    dst_in_win = (loc % P).astype(np.float32)
    # source mapping into quarter-major full layout
    sm = cols // local_real
    sl = cols - sm * local_real
    sw = sl // P
    rng = np.digitize(sw, WQ[1:NQ])              # 0..3
    idx_in_rng = sm * QROWS[rng] + (sl - P * np.array(WQ)[rng])
    assert idx_in_rng.max() < 32768

    counts = np.zeros((n_cores, n_windows, NQ), dtype=np.int64)
    np.add.at(counts, (core, win, rng), 1)
    # per-(window, range) chunk count: max over cores (SPMD shares one
    # instruction stream), 0 when empty on every core
    CH = np.ceil(counts.max(axis=0) / P).astype(np.int64)  # [n_windows, NQ]
    ngroups = n_windows * NQ
    slot = (CH.reshape(-1) * P).astype(np.int64)           # [ngroups]
    slot_start = np.concatenate([[0], np.cumsum(slot)])
    total = int(slot_start[-1])
    per_core = []
    for m in range(n_cores):
        sel = core == m
        key = win[sel] * np.int64(4 * 32768) + rng[sel] * np.int64(32768) + idx_in_rng[sel]
        order = np.argsort(key, kind="stable")
        mwin = win[sel][order]
        mrng = rng[sel][order]
        midx = idx_in_rng[sel][order].astype(np.int32)
        mw = w[sel][order].astype(np.float32)
        mdst = dst_in_win[sel][order]
        idx_arr = np.full(total, -1, dtype=np.int32)
        w_arr = np.zeros(total, dtype=np.float32)
        dst_arr = np.zeros(total, dtype=np.float32)
        gid = mwin * NQ + mrng
        group_counts = counts[m].reshape(-1)
        starts = np.zeros(ngroups + 1, dtype=np.int64)
        np.cumsum(group_counts, out=starts[1:])
        pos = np.arange(len(midx)) - starts[gid] + slot_start[gid]
        idx_arr[pos] = midx
        w_arr[pos] = mw
        dst_arr[pos] = mdst
        # per-group valid-index counts (gather descriptor trimming); msg
        # pool slots are zero-initialized once on device, so trimmed
        # gathers leave finite stale tails. Empty groups gather one row 0.
        gcnt = group_counts.copy()
        gcnt[gcnt == 0] = 1
        fill = np.repeat(np.arange(ngroups), slot)
        need = np.arange(total) < slot_start[fill] + gcnt[fill]
        idx_arr[need & (idx_arr < 0)] = 0
        # idx16 layout: per group [16, gp_r//16] with idx i at [i%16, i//16],
        # replicated across the 8 Q7-core partition groups; groups laid
        # consecutively -> window block width SG*8 int16 cols.
        blocks = []
        for g in range(ngroups):
            a = idx_arr[slot_start[g]:slot_start[g + 1]].reshape(-1, 16).T  # [16, gp/16]
            blocks.append(a)
        block = np.concatenate(blocks, axis=1).astype(np.int16)  # [16, SG*8*n_windows]
        idx16 = np.tile(block, (8, 1))
        # wv/dstv: [128, SG*n_windows], edge i of group g at [i%128, col g + i//128]
        def to_dev(arr):
            cols_ = []
            for g in range(ngroups):
                a = arr[slot_start[g]:slot_start[g + 1]].reshape(-1, P).T  # [128, gp/128]
                cols_.append(a)
            return np.ascontiguousarray(np.concatenate(cols_, axis=1).astype(np.float32))

        per_core.append(
            dict(idx16=np.ascontiguousarray(idx16), wv=to_dev(w_arr),
                 dstv=to_dev(dst_arr),
                 gcnt=np.ascontiguousarray(
                     gcnt.reshape(1, ngroups).astype(np.int32)))
        )
    return per_core, CH


def prep_weights(W, b, F, H, K=3):
    """W: [H, F*K] (torch-style interleaved k). Returns wt [K*F, H] bf16 with
    the Chebyshev recurrence folded in: row block 0: (W0 - W2)^T, block 1:
    W1^T, block 2: (2*W2)^T. Bias [1, H] bf16."""
    Wk = W.reshape(H, F, K).transpose(2, 1, 0)  # [k, f, h]
    wt = np.concatenate([Wk[0] - Wk[2], Wk[1], 2.0 * Wk[2]], axis=0)
    return np.ascontiguousarray(wt.astype(BF16)), np.ascontiguousarray(
        b.reshape(1, H).astype(BF16)
    )


# ---------------------------------------------------------------- device
class Builder:
    def __init__(self, nc, tc, cfg):
        self.nc = nc
        self.tc = tc
        self.cfg = cfg
        c = cfg
        self.n_windows = c["local_pad"] // P
        self.CH = c["CH"]                    # [n_windows, NQ] chunk counts
        # column prefix of group (w, r) in the wv/dstv/idx16 layouts
        flat = np.asarray(self.CH).reshape(-1)
        cs = np.concatenate([[0], np.cumsum(flat)])
        self.COL = cs[:-1].reshape(self.n_windows, NQ)
        self.WCOL = [int(cs[w * NQ]) for w in range(self.n_windows + 1)]
        self.QROWS = [(WQ[i + 1] - WQ[i]) * P for i in range(NQ)]
        WC = int(cs[-1])
        self.sb = tc.alloc_tile_pool(name="resident", bufs=1)
        self.dram = tc.alloc_tile_pool(name="dram", bufs=1, space="DRAM")
        # edge data residents (weights + dst-in-window); idx16 streamed
        self.w_res = self.sb.tile([P, WC], mybir.dt.float32, name="w_res")
        self.dst_res = self.sb.tile([P, WC], mybir.dt.float32, name="dst_res")
        wv_in = nc.dram_tensor("wv", [P, WC], mybir.dt.float32, kind="ExternalInput")
        dst_in = nc.dram_tensor("dstv", [P, WC], mybir.dt.float32, kind="ExternalInput")
        self.idx16_in = nc.dram_tensor(
            "idx16", [P, WC * 8], mybir.dt.int16, kind="ExternalInput"
        )
        NG = self.n_windows * NQ
        gcnt_in = nc.dram_tensor("gcnt", [1, NG], mybir.dt.int32,
                                 kind="ExternalInput")
        self.cnt_res = self.sb.tile([1, NG], mybir.dt.int32, name="cnt_res")
        nc.sync.dma_start(out=self.cnt_res[:], in_=gcnt_in[:, :])
        self.nregs = [nc.gpsimd.alloc_register(f"gtrim{i}") for i in range(8)]
        nc.sync.dma_start(out=self.w_res[:], in_=wv_in[:, :])
        nc.sync.dma_start(out=self.dst_res[:], in_=dst_in[:, :])
        # iota row tile [128, 128] bf16: value = column index
        iota_i = self.sb.tile([P, P], mybir.dt.int32, name="iota_i")
        nc.gpsimd.iota(iota_i[:], pattern=[[1, P]], base=0, channel_multiplier=0)
        self.iota_bf = self.sb.tile([P, P], mybir.dt.bfloat16, name="iota_bf")
        nc.vector.tensor_copy(self.iota_bf[:], iota_i[:])
        # per-partition index column [128, 1] for identity construction
        pid_i = self.sb.tile([P, 1], mybir.dt.int32, name="pid_i")
        nc.gpsimd.iota(pid_i[:], pattern=[[0, 1]], base=0, channel_multiplier=1)
        pid_f = self.sb.tile([P, 1], mybir.dt.float32, name="pid_f")
        nc.vector.tensor_copy(pid_f[:], pid_i[:])
        # identity and -0.5*identity [128, 128] bf16
        self.ident = self.sb.tile([P, P], mybir.dt.bfloat16, name="ident")
        nc.vector.tensor_scalar(
            out=self.ident[:], in0=self.iota_bf[:], scalar1=pid_f[:],
            scalar2=None, op0=mybir.AluOpType.is_equal,
        )
        # ones column for bias matmuls: [1, 128] bf16
        self.ones_row = self.sb.tile([1, P], mybir.dt.bfloat16, name="ones_row")
        nc.gpsimd.memset(self.ones_row[:], 1.0)
        # shared pools (one allocation for the whole net: no per-phase
        # SBUF reuse barriers, so adjacent phases pipeline freely)
        gmax = int(np.asarray(self.CH).max())
        msg_bufs = int(os.environ.get("CHEB_MSGBUFS", "7"))
        self.msgp = tc.alloc_tile_pool(name="msgp", bufs=msg_bufs)
        self.gmax = gmax
        # zero-fill every msg slot once: trimmed gathers only overwrite the
        # leading rows of a slot, and PE multiplies the stale tail by a zero
        # one-hot — stale bits must be finite, never uninitialized SBUF.
        for _ in range(msg_bufs):
            t = self.msgp.tile([P, gmax * 512], mybir.dt.bfloat16,
                               name="sp_m")
            nc.gpsimd.memset(t[:], 0.0)
        self.idxp = tc.alloc_tile_pool(name="idxp", bufs=3)
        self.ohp = tc.alloc_tile_pool(
            name="ohp", bufs=int(os.environ.get("CHEB_OHBUFS", "24")))
        self.spsp = tc.alloc_tile_pool(
            name="spsp", bufs=int(os.environ.get("CHEB_SPSBUFS", "5")),
            space="PSUM")
        self.epp = tc.alloc_tile_pool(name="epp", bufs=6)
        self.frp = tc.alloc_tile_pool(name="frp", bufs=6)
        self.atp = tc.alloc_tile_pool(
            name="atp", bufs=int(os.environ.get("CHEB_ATPBUFS", "24")))
        self.dpsp = tc.alloc_tile_pool(
            name="dpsp", bufs=int(os.environ.get("CHEB_DPSBUFS", "3")),
            space="PSUM")
        self.hp = tc.alloc_tile_pool(name="hp", bufs=6)
        self._gctr = 0

    def release_pools(self):
        for p in (self.hp, self.dpsp, self.atp, self.frp, self.epp, self.spsp,
                  self.ohp, self.idxp, self.msgp):
            p.release()

    def quarter_tensors(self, name, Fo, dt=mybir.dt.bfloat16, shared=False):
        mk = dict(addr_space="Shared") if shared else {}
        n = self.cfg["n_cores"] if shared else 1
        return [
            self.dram.tile([self.QROWS[q] * n, Fo], dt, name=f"{name}q{q}", **mk)
            for q in range(NQ)
        ]

    def allgather(self, loc_q, full_q):
        self.nc.gpsimd.collective_compute(
            "AllGather",
            mybir.AluOpType.bypass,
            replica_groups=[list(range(self.cfg["n_cores"]))],
            ins=[loc_q[:, :]],
            outs=[full_q[:, :]],
        )

    def spmm(self, srcs, F, name, out_quarters=None, out_single=None,
             fuse=None, bias=None, ag=None, out_scale=None, out_f32=False):
        """Per dst window w: psum = sum_e w_e * srcs[r][idx_e] over the 4
        ranges (+ optional fused matmuls), then write
        out = act_copy(psum [* out_scale]).

        srcs: 4 APs (range r rows x F). fuse: list of (lhsT_tile, rows_q)
        where rows_q is a 4-list of local quarter APs or a single AP —
        psum += lhsT^T @ rows[w]. bias: [1, F] tile. ag: callable(q) emitted
        after each quarter. out_scale: scale on the PSUM->SBUF copy."""
        nc = self.nc
        CH, COL, WCOL = self.CH, self.COL, self.WCOL
        SG8max = max(WCOL[w + 1] - WCOL[w] for w in range(self.n_windows)) * 8
        msgp, idxp, ohp, psp, epp = (
            self.msgp, self.idxp, self.ohp, self.spsp, self.epp)

        def window(w, qw):
            wl = w - WQ[qw]
            wch = [int(CH[w][r]) for r in range(NQ)]
            n_mm = (sum(wch) + (len(fuse) if fuse else 0)
                    + (1 if bias is not None else 0))
            if n_mm == 0:
                return
            wcols = WCOL[w + 1] - WCOL[w]
            psum = psp.tile([P, 512], mybir.dt.float32, name="sp_ps")[:, :F]
            idx_win = None
            if wcols:
                idx_win = idxp.tile([P, SG8max], mybir.dt.int16,
                                    name="sp_iw")[:, : wcols * 8]
                nc.sync.dma_start(
                    out=idx_win[:],
                    in_=self.idx16_in[:, bass.ds(WCOL[w] * 8, wcols * 8)],
                )
            mode = os.environ.get("CHEB_SPMM_MODE", "full")
            mm = 0
            for r in range(NQ):
                Gr = wch[r]
                if Gr == 0:
                    continue
                if mode in ("compute", "both"):
                    if not hasattr(self, "cmsg"):
                        self.cmsg = self.sb.tile(
                            [P, self.gmax * 512], mybir.dt.bfloat16,
                            name="cmsg")
                        nc.gpsimd.memset(self.cmsg[:], 0.0)
                    msg = self.cmsg[:, : Gr * F]
                else:
                    msg = msgp.tile([P, self.gmax * 512], mybir.dt.bfloat16,
                                    name="sp_m")[:, : Gr * F]
                roff = COL[w][r] - WCOL[w]
                if mode == "both":
                    gm = msgp.tile([P, self.gmax * 512], mybir.dt.bfloat16,
                                   name="sp_m")[:, : Gr * F]
                    nreg = self.nregs[self._gctr % 8]
                    nc.gpsimd.reg_load(
                        nreg, self.cnt_res[0:1, bass.ds(w * NQ + r, 1)],
                    )
                    nc.gpsimd.dma_gather(
                        out_ap=gm[:].rearrange("p (g f) -> p g f", g=Gr),
                        in_ap=srcs[r],
                        idxs_ap=idx_win[:, roff * 8: roff * 8 + Gr * 8],
                        num_idxs=Gr * P,
                        num_idxs_reg=nreg,
                        elem_size=F,
                        elem_step=F,
                        single_packet=False,
                        queue_num=self._gctr % NQUEUES,
                    )
                if mode not in ("compute", "both"):
                    nreg = self.nregs[self._gctr % 8]
                    nc.gpsimd.reg_load(
                        nreg, self.cnt_res[0:1, bass.ds(w * NQ + r, 1)],
                    )
                    nc.gpsimd.dma_gather(
                        out_ap=msg[:].rearrange("p (g f) -> p g f", g=Gr),
                        in_ap=srcs[r],
                        idxs_ap=idx_win[:, roff * 8: roff * 8 + Gr * 8],
                        num_idxs=Gr * P,
                        num_idxs_reg=nreg,
                        elem_size=F,
                        elem_step=F,
                        single_packet=bool(int(os.environ.get("CHEB_SP", "0"))),
                        queue_num=self._gctr % NQUEUES,
                    )
                self._gctr += 1
                if mode == "gather":
                    continue
                for c in range(Gr):
                    col_s = bass.ds(int(COL[w][r]) + c, 1)
                    oh = ohp.tile([P, P], mybir.dt.bfloat16, name="sp_oh")
                    nc.vector.tensor_scalar(
                        out=oh[:],
                        in0=self.iota_bf[:],
                        scalar1=self.dst_res[:, col_s],
                        scalar2=self.w_res[:, col_s],
                        op0=mybir.AluOpType.is_equal,
                        op1=mybir.AluOpType.mult,
                    )
                    nc.tensor.matmul(
                        out=psum[:],
                        lhsT=oh[:],
                        rhs=msg[:, c * F: (c + 1) * F],
                        start=(mm == 0),
                        stop=(mm == n_mm - 1),
                    )
                    mm += 1
            if mode == "gather":
                return
            if fuse:
                for lhsT, rows_q in fuse:
                    if isinstance(rows_q, list):
                        src_ap = rows_q[qw][bass.ds(wl * P, P), :F]
                    else:
                        src_ap = rows_q[bass.ds(w * P, P), :F]
                    rt = self.frp.tile([P, 512], mybir.dt.bfloat16,
                                       name="sp_fr")[:, :F]
                    nc.sync.dma_start(out=rt[:], in_=src_ap)
                    nc.tensor.matmul(
                        out=psum[:], lhsT=lhsT[:], rhs=rt[:],
                        start=False, stop=(mm == n_mm - 1),
                    )
                    mm += 1
            if bias is not None:
                nc.tensor.matmul(
                    out=psum[:], lhsT=self.ones_row[:], rhs=bias[:],
                    start=False, stop=True,
                )
                mm += 1
            out_dt = mybir.dt.float32 if out_f32 else mybir.dt.bfloat16
            ysb = epp.tile([P, 512], out_dt, name="sp_y")[:, :F]
            kw = dict(scale=out_scale) if out_scale is not None else {}
            nc.scalar.activation(
                ysb[:], psum[:], mybir.ActivationFunctionType.Copy, **kw
            )
            if out_quarters is not None:
                nc.sync.dma_start(
                    out=out_quarters[qw][bass.ds(wl * P, P), :], in_=ysb[:]
                )
            else:
                nc.sync.dma_start(
                    out=out_single[bass.ds(w * P, P), :], in_=ysb[:]
                )

        for qw in range(NQ):
            for w in range(WQ[qw], WQ[qw + 1]):
                window(w, qw)
            if ag is not None:
                ag(qw)

    def dense(self, acts, act_widths, heads, name):
        """For each 512-row group g: load transposed act tiles, then per
        128-row sub-window n accumulate each head's psum over all (s, k)
        chunks (+ bias), apply relu/copy, store.

        acts: list of S sources (4-list of quarter APs or single AP).
        heads: list of dicts: wt (list of [128, Fo] tiles, one per (s, k)),
        bias (tile or None), relu, out_f32, out (4-list quarter APs or
        single AP), ag (callable(q) or None), Fo."""
        nc = self.nc
        atp, psp, hp = self.atp, self.dpsp, self.hp
        n_groups = self.cfg["local_pad"] // 512

        for g in range(n_groups):
            q = next(i for i in range(NQ) if GB[i + 1] > g)
            gl = g - GB[q]
            at_tiles = []
            for s, src in enumerate(acts):
                Fs = act_widths[s]
                for k in range(Fs // P):
                    at = atp.tile([P, 512], mybir.dt.bfloat16, name="dn_at")
                    if isinstance(src, list):
                        src_ap = src[q][bass.ds(gl * 512, 512), k * P:(k + 1) * P]
                    else:
                        src_ap = src[bass.ds(g * 512, 512), k * P:(k + 1) * P]
                    nc.sync.dma_start_transpose(out=at[:], in_=src_ap)
                    at_tiles.append(at)
            for n in range(4):
                outs = []
                for h in heads:
                    Fo = h["Fo"]
                    psum = psp.tile([P, 512], mybir.dt.float32,
                                    name="dn_ps")[:, :Fo]
                    n_mm = len(at_tiles) + (1 if h["bias"] is not None else 0)
                    for i, at in enumerate(at_tiles):
                        nc.tensor.matmul(
                            out=psum[:],
                            lhsT=at[:, n * P:(n + 1) * P],
                            rhs=h["wt"][i][:],
                            start=(i == 0),
                            stop=(i == n_mm - 1),
                        )
                    if h["bias"] is not None:
                        nc.tensor.matmul(
                            out=psum[:], lhsT=self.ones_row[:],
                            rhs=h["bias"][:], start=False, stop=True,
                        )
                    out_dt = (mybir.dt.float32 if h.get("out_f32")
                              else mybir.dt.bfloat16)
                    ht = hp.tile([P, 512], out_dt, name="dn_h")[:, :Fo]
                    nc.scalar.activation(
                        ht[:], psum[:],
                        mybir.ActivationFunctionType.Relu if h["relu"]
                        else mybir.ActivationFunctionType.Copy,
                    )
                    outs.append((h, ht))
                for h, ht in outs:
                    if isinstance(h["out"], list):
                        nc.sync.dma_start(
                            out=h["out"][q][bass.ds((gl * 4 + n) * P, P), :],
                            in_=ht[:],
                        )
                    else:
                        nc.sync.dma_start(
                            out=h["out"][bass.ds((g * 4 + n) * P, P), :],
                            in_=ht[:],
                        )
            if g + 1 in GB:
                for h in heads:
                    if h.get("ag") is not None:
                        h["ag"](q)

    def load_weights(self, wt_dram, bias_dram, F, H, name):
        """Load [K*F, H] weight into 3*F//P resident sbuf tiles + bias row."""
        nc = self.nc
        tiles = []
        for i in range(3 * F // P):
            t = self.sb.tile([P, H], mybir.dt.bfloat16, name=f"{name}_w{i}")
            nc.sync.dma_start(out=t[:], in_=wt_dram[i * P:(i + 1) * P, :])
            tiles.append(t)
        b = self.sb.tile([1, H], mybir.dt.bfloat16, name=f"{name}_b")
        nc.sync.dma_start(out=b[:], in_=bias_dram[:, :])
        return tiles, b


def build(cfg):
    nc = bacc.Bacc(
        "TRN2",
        target_bir_lowering=False,
        debug=False,
        num_devices=cfg["n_cores"],
        num_swdge_queues=NQUEUES,
    )
    F_IN, H, F_OUT = cfg["F_IN"], cfg["H"], cfg["F_OUT"]
    lp, nf = cfg["local_pad"], cfg["n_full"]

    xfull = nc.dram_tensor("xfull", [nf, F_IN], mybir.dt.bfloat16, kind="ExternalInput")
    xloc = nc.dram_tensor("xloc", [lp, F_IN], mybir.dt.bfloat16, kind="ExternalInput")
    wts = {}
    dims = [(F_IN, H), (H, H), (H, H), (H, F_OUT)]
    for i, (F, Ho) in enumerate(dims):
        wts[i] = (
            nc.dram_tensor(f"wt{i}", [3 * F, Ho], mybir.dt.bfloat16, kind="ExternalInput"),
            nc.dram_tensor(f"bias{i}", [1, Ho], mybir.dt.bfloat16, kind="ExternalInput"),
        )
    out_ext = nc.dram_tensor("out", [lp, F_OUT], mybir.dt.float32, kind="ExternalOutput")

    with tile.TileContext(nc) as tc:
        b = Builder(nc, tc, cfg)
        w_res = {i: b.load_weights(wts[i][0], wts[i][1], F, Ho, f"L{i}")
                 for i, (F, Ho) in enumerate(dims)}
        RS = [0]
        for q in range(NQ):
            RS.append(RS[-1] + b.QROWS[q] * cfg["n_cores"])
        x_srcs = [xfull[RS[r]:RS[r + 1], :] for r in range(NQ)]

        stage = os.environ.get("CHEB_STAGE", "")
        if stage == "ag":
            # 8 AllGather sets, quarter-chunked, chained via shared tensors
            widths = [256, 512, 512, 512, 512, 512, 256, 256]
            locs = {w: b.quarter_tensors(f"agl{w}", w) for w in (256, 512)}
            seed = b.sb.tile([P, 512], mybir.dt.bfloat16, name="agseed")
            nc.gpsimd.memset(seed[:], 0.0)
            for q in range(NQ):
                for w in (256, 512):
                    nc.sync.dma_start(out=locs[w][q][0:P, :], in_=seed[:, :w])
            for j, w in enumerate(widths):
                fulls = b.quarter_tensors(f"agf{j}", w, shared=True)
                for q in range(NQ):
                    b.allgather(locs[w][q], fulls[q])
            b.release_pools()
            b.sb.release()
            b.dram.release()
            return nc
        if stage == "agspmm":
            # independent AG chain + spmm passes: measures DMA/collective overlap
            locs = {w: b.quarter_tensors(f"agl{w}", w) for w in (256, 512)}
            seed = b.sb.tile([P, 512], mybir.dt.bfloat16, name="agseed")
            nc.gpsimd.memset(seed[:], 0.0)
            for q in range(NQ):
                for w in (256, 512):
                    nc.sync.dma_start(out=locs[w][q][0:P, :], in_=seed[:, :w])
            widths_ag = [256, 512, 512, 512, 512, 512, 256, 256]
            ag_jobs = []
            for j, w in enumerate(widths_ag):
                fulls = b.quarter_tensors(f"agf{j}", w, shared=True)
                for q in range(NQ):
                    ag_jobs.append((locs[w][q], fulls[q]))
            widths = [256, 256, 512, 512, 512, 512, 256, 256]
            src512 = b.quarter_tensors("gsrc", 512, shared=True)
            outs = {w: b.quarter_tensors(f"spo{w}", w) for w in (256, 512)}
            ji = [0]

            def agk(q, _=None):
                if ji[0] < len(ag_jobs):
                    b.allgather(*ag_jobs[ji[0]])
                    ji[0] += 1

            for j, w in enumerate(widths):
                srcs = x_srcs if w == 256 else [src512[r][:, :]
                                                for r in range(NQ)]
                b.spmm(srcs, w, f"dbg_sp{j}", out_quarters=outs[w],
                       ag=lambda q, _j=j: agk(q))
            b.release_pools()
            b.sb.release()
            b.dram.release()
            return nc
        if stage == "spmm8":
            widths = [256, 256, 512, 512, 512, 512, 256, 256]
            src512 = b.quarter_tensors("gsrc", 512, shared=True)
            outs = {w: b.quarter_tensors(f"spo{w}", w) for w in (256, 512)}
            for j, w in enumerate(widths):
                srcs = x_srcs if w == 256 else [src512[r][:, :]
                                                for r in range(NQ)]
                b.spmm(srcs, w, f"dbg_sp{j}", out_quarters=outs[w])
            b.release_pools()
            b.sb.release()
            b.dram.release()
            return nc
        if stage == "sp1":
            b.spmm(x_srcs, F_IN, "dbg_sp1", out_single=out_ext, out_f32=True)
            b.release_pools()
            b.sb.release()
            b.dram.release()
            return nc
        if stage == "sp2":
            x1q = b.quarter_tensors("dx1", F_IN)
            x1f = b.quarter_tensors("dx1f", F_IN, shared=True)
            b.spmm(x_srcs, F_IN, "dbg_sp1", out_quarters=x1q,
                   ag=lambda q: b.allgather(x1q[q], x1f[q]))
            b.spmm([x1f[r][:, :] for r in range(NQ)], F_IN, "dbg_sp2",
                   out_single=out_ext, out_f32=True)
            b.release_pools()
            b.sb.release()
            b.dram.release()
            return nc

        act_srcs, act_loc, act_w = x_srcs, xloc, F_IN
        for i in range(3):
            F, Ho = dims[i]
            x1q = b.quarter_tensors(f"x1l{i}", F)
            x1f = b.quarter_tensors(f"x1f{i}", F, shared=True)
            x2q = b.quarter_tensors(f"x2l{i}", F)
            hq = b.quarter_tensors(f"hl{i}", Ho)
            hf = b.quarter_tensors(f"hf{i}", Ho, shared=True)
            b.spmm(act_srcs, F, f"sp1_{i}", out_quarters=x1q,
                   ag=lambda q, _l=x1q, _f=x1f: b.allgather(_l[q], _f[q]))
            x1_srcs = [x1f[r][:, :] for r in range(NQ)]
            # folded weights: dense consumes y2 = L x1 directly (block 2 is
            # (2 W2)^T and block 0 absorbs -W2), so no 2*y - x combine here.
            b.spmm(x1_srcs, F, f"sp2_{i}", out_quarters=x2q)
            acts_d = [act_loc if i == 0 else act_qlist, x1q, x2q]
            b.dense(acts_d, [F, F, F],
                    [dict(wt=w_res[i][0], bias=w_res[i][1], relu=True,
                          out=hq, Fo=Ho,
                          ag=lambda q, _l=hq, _f=hf: b.allgather(_l[q], _f[q]))],
                    f"dense{i}")
            act_srcs = [hf[r][:, :] for r in range(NQ)]
            act_qlist = hq
            act_w = Ho

        # ---- layer 4, project-first: y = h A + L(h B + L(h C)) ----
        F, Fo = dims[3]          # 512 -> 256
        wt4 = w_res[3][0]        # 12 tiles: A = 0:4, B = 4:8, C = 8:12
        A_t, B_t, C_t = wt4[0:4], wt4[4:8], wt4[8:12]
        qd = b.quarter_tensors("qd", Fo)
        qf = b.quarter_tensors("qf", Fo, shared=True)
        uq = b.quarter_tensors("ul", Fo)
        uf = b.quarter_tensors("uf", Fo, shared=True)
        hBq = b.quarter_tensors("hB", Fo)
        hAq = b.quarter_tensors("hA", Fo)
        b.dense([act_qlist], [F],
                [dict(wt=C_t, bias=None, relu=False, out=qd, Fo=Fo,
                      ag=lambda q: b.allgather(qd[q], qf[q])),
                 dict(wt=B_t, bias=None, relu=False, out=hBq, Fo=Fo),
                 dict(wt=A_t, bias=None, relu=False, out=hAq, Fo=Fo)],
                "proj4")
        q_srcs = [qf[r][:, :] for r in range(NQ)]
        b.spmm(q_srcs, Fo, "sp1_3", out_quarters=uq,
               fuse=[(b.ident, hBq)],
               ag=lambda q: b.allgather(uq[q], uf[q]))
        u_srcs = [uf[r][:, :] for r in range(NQ)]
        b.spmm(u_srcs, Fo, "sp2_3", out_single=out_ext,
               fuse=[(b.ident, hAq)], bias=w_res[3][1], out_f32=True)

        b.release_pools()
        b.sb.release()
        b.dram.release()
    return nc


# ---------------------------------------------------------------- top level
def run(x, edge_rows, edge_cols, edge_weight, Ws, bs, n_cores=8, trace=False):
    """Ws/bs: lists of 4 (W, b) numpy arrays. Returns [N, F_OUT] f32."""
    from concourse.bass_utils import run_bass_kernel_spmd

    N = x.shape[0]
    F_IN = x.shape[1]
    H = Ws[1].shape[0]
    F_OUT = Ws[3].shape[0]
    assert N % n_cores == 0
    local_real = N // n_cores
    local_pad = ((local_real + 511) // 512) * 512
    n_windows = local_pad // P
    n_full = local_pad * n_cores

    per_core, CH = prep_edges(
        edge_rows, edge_cols, edge_weight, n_cores, local_real, local_pad,
        n_windows
    )
    # quarter-major padded full x layout
    QROWS = [(WQ[i + 1] - WQ[i]) * P for i in range(NQ)]
    xb = np.asarray(x).astype(BF16)
    xp = np.zeros((n_full, F_IN), dtype=BF16)
    xl = np.zeros((n_cores, local_pad, F_IN), dtype=BF16)
    rs = 0
    for q in range(NQ):
        for m in range(n_cores):
            lo = WQ[q] * P
            hi = min(WQ[q + 1] * P, local_real)
            if hi > lo:
                seg = xb[m * local_real + lo: m * local_real + hi]
                xp[rs + m * QROWS[q]: rs + m * QROWS[q] + (hi - lo)] = seg
        rs += QROWS[q] * n_cores
    for m in range(n_cores):
        xl[m, :local_real] = xb[m * local_real: (m + 1) * local_real]

    dims = [(F_IN, H), (H, H), (H, H), (H, F_OUT)]
    wt_np = {}
    for i, (F, Ho) in enumerate(dims):
        wt, bias = prep_weights(Ws[i], bs[i], F, Ho)
        wt_np[f"wt{i}"] = wt
        wt_np[f"bias{i}"] = bias

    cfg = dict(
        n_cores=n_cores, F_IN=F_IN, H=H, F_OUT=F_OUT,
        local_real=local_real, local_pad=local_pad, n_full=n_full, CH=CH,
    )
    nc = build(cfg)
    if not nc.is_finalized():
        nc.finalize()
    in_maps = []
    for m in range(n_cores):
        im = dict(
            xfull=xp,
            xloc=np.ascontiguousarray(xl[m]),
            idx16=per_core[m]["idx16"],
            wv=per_core[m]["wv"],
            dstv=per_core[m]["dstv"],
            gcnt=per_core[m]["gcnt"],
            **wt_np,
        )
        in_maps.append(im)
    if trace == "timed":
        import timed_exec

        results, times = timed_exec.timed_run(nc, in_maps, n_cores)
        out = np.concatenate(
            [results[m]["out"][:local_real] for m in range(n_cores)], axis=0
        )
        return out, times
    res = run_bass_kernel_spmd(
        nc, in_maps, core_ids=list(range(n_cores)), trace=trace
    )
    out = np.concatenate(
        [res.results[m]["out"][:local_real] for m in range(n_cores)], axis=0
    )
    return out, res


# ---------------------------------------------------------------- entry
def kernel(x, edge_rows, edge_cols, edge_weight, W1, b1, W2, b2, W3, b3,
           Wout, bout):
    Ws = [np.asarray(W1), np.asarray(W2), np.asarray(W3), np.asarray(Wout)]
    bs = [np.asarray(b1), np.asarray(b2), np.asarray(b3), np.asarray(bout)]
    out, _ = run(
        np.asarray(x), np.asarray(edge_rows), np.asarray(edge_cols),
        np.asarray(edge_weight), Ws, bs, n_cores=8, trace=False,
    )
    return out.astype(np.float32)
